# revision 9
# baseline (speedup 1.0000x reference)
"""Single-head attention (B=4, S=4096, D=1024, N=L=128) on 8 trn2 NeuronCores.

Sharding: core c handles batch b = c//2, query half h = c%2 (2048 queries).
Each core receives the full context of its batch with its own query half
ordered FIRST (attention is permutation-invariant over the context axis).

fp8 strategy (per-stage, validated numerically against the f64 reference):
  - Projections run as fp8 DoubleRow matmuls (0.5 cycles/out-col, 2 planes
    of 128 contraction each) in THREE passes: x_hi@W32h (+x_lo plane fused),
    then x_hi@W32l over d-tile pairs. W32 = 32*W is pre-scaled on the host so
    its fp8 encoding avoids the e4m3 subnormal floor (sigma_W = 1/32); the
    32x is folded into the exp scale (q,k) / final epilogue multiply (v).
    Host supplies x as interleaved fp8 (hi, lo) residual pairs, so a proj
    chunk is 8 DR matmuls (pass12: planes = (x_hi_d, x_lo_d) vs duplicated
    W32h_d) + 4 DR matmuls (pass3: planes = d-tile pairs of x_hi vs W32l).
    12*256 cycles vs bf16's 8*512: 25% cheaper at bf16-level accuracy.
  - Scores run as fp8 DoubleRow with stationary [k_hi | k_hi] (stride-0
    plane broadcast) and moving [q_hi | q_lo]: full-precision q times fp8 k
    at 2x bf16 rate. Only the single k quantization (~2.4% rms) enters the
    softmax logits; measured end-to-end rel err ~9e-3 (gate 2e-2).
  - exp on ACT with scale = 1/(sqrt(D)*1024) (q,k both carry 32x).
  - PV stays bf16 (fp8 on either side measured 2-3e-2: over the gate).
Per-engine busy (cost model): PE ~69us, ACT (exp) ~66us, DVE ~55us.

Per-core pipeline (single interleaved emission). The schedule balances PE
against the ACT exp stream (64 x [128,1024] exps ~= 66us, the hard floor):
half 0 runs proj + scores only (PE ~34us ~= ACT), deferring ALL of half 0's
PV matmuls; half 1 runs scores + two PV groups per step (h0's deferred PVs
drain first, then h1's own) so PE stays fed at ~ACT rate throughout. All
half-0 sT tiles stay live in SBUF until their deferred PVs consume them.
PSUM: proj(2 banks)+pss(4) in half 0; pss(4)+po0(2)+po1(2) in half 1 (proj
pool released first); the denominator fold lands in pss-tagged tiles at the
tail. Softmax denominator partials accumulate as bf16 adds on DVE (chains
0-2) / gpsimd (chain 3, SBUF only - gpsimd cannot touch PSUM); chains
pre-fold on DVE then one ones-matmul broadcasts the denominator row across
partitions; epilogue does DVE reciprocal then a single scalar_tensor_tensor
(po * 1/32) * recip before the output DMA.
"""
from contextlib import ExitStack

import numpy as np
import ml_dtypes

import concourse.tile as tile
import concourse.mybir as mybir
from concourse import bacc
from concourse.bass_utils import run_bass_kernel_spmd

B, S, D, N, L = 4, 4096, 1024, 128, 128
NCORES = 8
SQ = B * S // NCORES      # 2048 queries per core
CCH = 512                 # projection chunk (tokens)
NCH = S // CCH            # 8 projection chunks
NKC = S // 128            # 32 kctx subchunks of 128
QH = 1024                 # query half processed per attention sweep
ND = D // 128             # 8 contraction tiles over D
NCHAIN = 4                # denominator partial chains
SCALE = 1.0 / float(np.sqrt(D))
EXP_SCALE = SCALE / 1024.0   # q,k each carry a 32x from W32 host pre-scale

BF = mybir.dt.bfloat16
F32 = mybir.dt.float32
F8 = mybir.dt.float8e4
E4 = ml_dtypes.float8_e4m3

DR = mybir.MatmulPerfMode.DoubleRow

# xt8 element offsets: [p, (c d two s)], two = (hi, lo)
XCH = ND * 2 * CCH        # elems per chunk per partition (8192)


def emit(nc, tc, ctx, xt, wcat, out):
    persist = ctx.enter_context(tc.tile_pool(name="persist", bufs=1))
    zwarm = persist.tile([128, 128], BF, tag="zwarm")
    nc.gpsimd.memset(zwarm, 0.0)
    ones_mat = persist.tile([128, 128], BF, tag="ones_mat")
    nc.vector.memset(ones_mat, 1.0)

    xbig = persist.tile([128, NCH * XCH], F8, tag="xbig")
    # wcat: W32h [p, (m d n)] then W32l [p, (m d n)], m = (q, k, v)
    WSZ = 3 * ND * N
    wsb = persist.tile([128, 2 * WSZ], F8, tag="wsb")

    def xdma(c, parts=1):
        w = XCH // parts
        for s in range(parts):
            sl = slice(c * XCH + s * w, c * XCH + (s + 1) * w)
            nc.sync.dma_start(out=xbig[:, sl], in_=xt[:, sl])

    # Wq/Wk first (pieces q(0), k(0), q(1) gate the first score group), then
    # chunk 0 in two halves so proj(0) starts as soon as the first half
    # lands, then chunk 1, then Wv.
    W1 = ND * N
    half = XCH // 2

    def wdma(m):
        nc.sync.dma_start(out=wsb[:, m * W1:(m + 1) * W1],
                          in_=wcat[:, m * W1:(m + 1) * W1])
        nc.sync.dma_start(out=wsb[:, WSZ + m * W1:WSZ + (m + 1) * W1],
                          in_=wcat[:, WSZ + m * W1:WSZ + (m + 1) * W1])

    wdma(0)
    nc.sync.dma_start(out=xbig[:, 0:half], in_=xt[:, 0:half])
    wdma(1)
    nc.sync.dma_start(out=xbig[:, half:2 * half], in_=xt[:, half:2 * half])
    xdma(1)
    wdma(2)
    for c in range(2, NCH):
        xdma(c)

    def wh2(m, d):
        # stationary [128, 2, 128]: duplicated W32h_d planes (stride-0)
        w = wsb[:, (m * ND + d) * N:(m * ND + d + 1) * N]
        return w.unsqueeze(1).broadcast_to((128, 2, N))

    def wl2(m, t):
        # stationary [128, 2, 128]: planes (W32l_{2t}, W32l_{2t+1})
        sl = slice(WSZ + (m * ND + 2 * t) * N, WSZ + (m * ND + 2 * t + 2) * N)
        return wsb[:, sl].rearrange("p (two n) -> p two n", two=2)

    def x12(c, d):
        # moving [128, 2, 512]: planes (x_hi_d, x_lo_d), contiguous
        off = c * XCH + d * 2 * CCH
        return xbig[:, off:off + 2 * CCH].rearrange(
            "p (two s) -> p two s", two=2)

    def x3(c, t):
        # moving [128, 2, 512]: planes (x_hi_{2t}, x_hi_{2t+1}), d-stride 1024
        off = c * XCH + 2 * t * 2 * CCH
        return xbig[:, off:off + 3 * CCH].rearrange(
            "p (d s) -> p d s", d=3)[:, 0::2, :]

    def proj_mm(ps, m, c):
        for d in range(ND):
            nc.tensor.matmul(ps, wh2(m, d), x12(c, d),
                             start=(d == 0), stop=False, perf_mode=DR)
        for t in range(ND // 2):
            nc.tensor.matmul(ps, wl2(m, t), x3(c, t),
                             start=False, stop=(t == ND // 2 - 1),
                             perf_mode=DR)

    kT8 = persist.tile([128, S], F8, tag="kT8")     # [n, kctx] fp8 (32x)
    vv = persist.tile([128, S], BF, tag="vv")       # 32 chunks [kctx128, l]
    qhl = persist.tile([128, 2 * SQ], F8, tag="qhl")  # per 512-q: (hi, lo)

    vtc_pool = ctx.enter_context(tc.tile_pool(name="vtc", bufs=3))

    # ---------------- attention pools (outer; proj pool nests inside) ----
    spool = ctx.enter_context(tc.tile_pool(name="sT", bufs=1))
    parts = ctx.enter_context(tc.tile_pool(name="parts", bufs=1))
    pss_pool = ctx.enter_context(tc.tile_pool(name="pss", bufs=2, space="PSUM"))
    po_pool = ctx.enter_context(tc.tile_pool(name="po", bufs=1, space="PSUM"))
    epi_sb = ctx.enter_context(tc.tile_pool(name="episb", bufs=2))

    proj_cm = tc.tile_pool(name="proj", bufs=2, space="PSUM")
    proj_ps = proj_cm.__enter__()

    # PE warmup while the first DMAs land (p-state ramp off critical path)
    for _ in range(30):
        pwarm = proj_ps.tile([128, CCH], F32, tag="proj", name="pwarm")
        nc.tensor.matmul(pwarm[:, 0:128], zwarm, zwarm, start=True,
                         stop=True)

    def piece_k(c):
        csl = slice(c * CCH, (c + 1) * CCH)
        pk = proj_ps.tile([128, CCH], F32, tag="proj", name="pk")
        proj_mm(pk, 1, c)
        nc.vector.tensor_copy(kT8[:, csl], pk)

    vt_pending = []

    def flush_vt(n=None):
        while vt_pending and (n is None or len(vt_pending) > n):
            csl, vTc = vt_pending.pop(0)
            nc.sync.dma_start_transpose(
                out=vv[:, csl].rearrange("p (t q) -> p t q", t=CCH // 128),
                in_=vTc)

    def piece_v(c):
        csl = slice(c * CCH, (c + 1) * CCH)
        pv = proj_ps.tile([128, CCH], F32, tag="proj", name="pv")
        proj_mm(pv, 2, c)
        vTc = vtc_pool.tile([128, CCH], BF, tag="vTc", name=f"vTc{c % 3}")
        nc.vector.tensor_copy(vTc, pv)
        # defer the XBAR transpose issue one piece so it queues behind the
        # next x chunk's transfer on the serialized DMA engines
        vt_pending.append((csl, vTc))
        flush_vt(1)

    def piece_q(c):
        pq = proj_ps.tile([128, CCH], F32, tag="proj", name="pq")
        proj_mm(pq, 0, c)
        hi = slice(c * 2 * CCH, c * 2 * CCH + CCH)
        lo = slice(c * 2 * CCH + CCH, (c + 1) * 2 * CCH)
        nc.vector.tensor_copy(qhl[:, hi], pq)
        nc.vector.tensor_tensor(out=qhl[:, lo], in0=pq, in1=qhl[:, hi],
                                op=mybir.AluOpType.subtract)

    po = [None, None]
    part = {}
    sTs = {}

    def emit_attn_score(h, i):
        ksl = slice(i * 128, (i + 1) * 128)
        kst = kT8[:, ksl].unsqueeze(1).broadcast_to((128, 2, 128))
        pss = pss_pool.tile([128, QH], F32, tag="pss")
        for j in range(2):
            blk = h * 2 + j
            qmv = qhl[:, blk * 2 * CCH:(blk + 1) * 2 * CCH].rearrange(
                "p (two s) -> p two s", two=2)
            nc.tensor.matmul(pss[:, j * (QH // 2):(j + 1) * (QH // 2)],
                             kst, qmv, start=True, stop=True,
                             perf_mode=DR, skip_group_check=True)
        # h0 sT tiles all stay live until their deferred PVs run in half 1;
        # h1 reuses h0's tags as the deferred PVs free them (2 per h1 step:
        # even tags over steps 1-16, odd tags after), so the pool is 33 bufs
        if h == 0:
            tg = f"sTa{i}"
        elif i == 0:
            tg = "sTb0"
        elif i <= 16:
            tg = f"sTa{2 * (i - 1)}"
        else:
            tg = f"sTa{2 * (i - 17) + 1}"
        sT = spool.tile([128, QH], BF, tag=tg, name=tg)
        nc.scalar.activation(sT, pss, func=mybir.ActivationFunctionType.Exp,
                             scale=EXP_SCALE)
        sTs[h, i] = sT
        ch = i % NCHAIN
        eng = nc.gpsimd if (ch == 3 and i < NKC - 8) else nc.vector
        if i < NCHAIN:
            part[h, ch] = parts.tile([128, QH], BF, tag=f"part{h}_{ch}",
                                     name=f"part{h}_{ch}")
            eng.tensor_copy(part[h, ch], sT)
        elif i == NKC - 1:
            # column-split the final add so the j0 fold/reciprocal chain can
            # start half an add earlier
            for j in range(2):
                jsl = slice(j * (QH // 2), (j + 1) * (QH // 2))
                eng.tensor_add(part[h, ch][:, jsl], part[h, ch][:, jsl],
                               sT[:, jsl])
        else:
            eng.tensor_add(part[h, ch], part[h, ch], sT)

    def emit_attn_pv(h, i):
        ksl = slice(i * 128, (i + 1) * 128)
        if i == 0:
            po[h] = po_pool.tile([128, QH], F32, tag="po", name=f"po{h}")
        for j in range(2):
            jsl = slice(j * (QH // 2), (j + 1) * (QH // 2))
            nc.tensor.matmul(po[h][:, jsl], vv[:, ksl], sTs[h, i][:, jsl],
                             start=(i == 0), stop=(i == NKC - 1),
                             skip_group_check=True)

    prsR = [None, None]

    def emit_fold_pre(h, j):
        # collapse chains 1..3 into chain 0 on DVE, one column half at a time
        jsl = slice(j * (QH // 2), (j + 1) * (QH // 2))
        for ch in range(1, NCHAIN):
            nc.vector.tensor_add(part[h, 0][:, jsl], part[h, 0][:, jsl],
                                 part[h, ch][:, jsl])

    def emit_fold_mm(h, j):
        # ones-matmul broadcasts the denominator row to every partition;
        # output lands in a pss-tagged tile (free at the tail)
        if j == 0:
            prsR[h] = pss_pool.tile([128, QH], F32, tag="pss",
                                    name=f"prsR{h}")
        jsl = slice(j * (QH // 2), (j + 1) * (QH // 2))
        nc.tensor.matmul(prsR[h][:, jsl], ones_mat, part[h, 0][:, jsl],
                         start=True, stop=True, skip_group_check=True)

    def emit_epi_finish(h):
        for j in range(2):
            jsl = slice(j * (QH // 2), (j + 1) * (QH // 2))
            recipB = epi_sb.tile([128, QH // 2], F32, tag="recipB")
            nc.vector.reciprocal(recipB, prsR[h][:, jsl])
            foall = epi_sb.tile([128, QH // 2], F32, tag="foall",
                                name=f"foall{j}")
            # v carries a 32x from W32v: out = (po * 1/32) * recip
            nc.vector.scalar_tensor_tensor(
                out=foall, in0=po[h][:, jsl], scalar=1.0 / 32.0, in1=recipB,
                op0=mybir.AluOpType.mult, op1=mybir.AluOpType.mult)
            nc.sync.dma_start(out=out[:, h * QH + j * (QH // 2):
                                      h * QH + (j + 1) * (QH // 2)],
                              in_=foall)

    # ---------------- schedule ----------------
    # Half 0: proj pieces woven into the 32 score-only steps; q(0),k(0),q(1)
    # run up front so the first exp lands as early as possible. k(c) is
    # hard-gated before step 4c.
    kv_done = {"k": 0}

    def mk_k(c):
        def f():
            piece_k(c)
            kv_done["k"] = c + 1
        return f

    rest = [mk_k(1), lambda: piece_v(0), lambda: piece_v(1),
            mk_k(2), lambda: piece_v(2), mk_k(3), lambda: piece_q(2),
            lambda: piece_v(3), mk_k(4), lambda: piece_q(3),
            lambda: piece_v(4), mk_k(5), lambda: piece_v(5),
            mk_k(6), lambda: piece_v(6), mk_k(7), lambda: piece_v(7)]

    piece_q(0)
    mk_k(0)()
    piece_q(1)
    emitted = 0
    for i in range(NKC):
        need_k = i // 4 + 1
        while emitted < len(rest) and (
                kv_done["k"] < need_k
                or emitted * 32 < len(rest) * (i + 1)):
            rest[emitted]()
            emitted += 1
        emit_attn_score(0, i)
    while emitted < len(rest):
        rest[emitted]()
        emitted += 1
    flush_vt(0)
    proj_cm.__exit__(None, None, None)

    # Half 1: two PV groups per score step — h0's deferred PVs first, then
    # h1's own (trailing their exps by >= 2 steps).
    for j in range(2):
        emit_fold_pre(0, j)
    pv0 = list(range(NKC))
    h1n = {"n": 0}

    def pop_pv():
        if pv0:
            emit_attn_pv(0, pv0.pop(0))
        elif h1n["n"] < NKC - 1:
            emit_attn_pv(1, h1n["n"])
            h1n["n"] += 1

    for s in range(NKC):
        emit_attn_score(1, s)
        for _ in range(2):
            if pv0 or h1n["n"] < min(s, NKC - 1):
                pop_pv()
    while pv0 or h1n["n"] < NKC - 1:
        pop_pv()
    emit_fold_mm(0, 0)
    emit_fold_mm(0, 1)
    emit_attn_pv(1, NKC - 1)
    emit_epi_finish(0)
    for j in range(2):
        emit_fold_pre(1, j)
        emit_fold_mm(1, j)
    emit_epi_finish(1)


def build_bass(iters=1):
    nc = bacc.Bacc()
    xt = nc.dram_tensor("xt_part", [128, NCH * XCH], F8,
                        kind="ExternalInput")
    wcat = nc.dram_tensor("wcat", [128, 2 * 3 * ND * N], F8,
                          kind="ExternalInput")
    out = nc.dram_tensor("out_part", [128, (SQ // 128) * L], F32,
                         kind="ExternalOutput")
    with tile.TileContext(nc) as tc:
        for _ in range(iters):
            with ExitStack() as ctx:
                emit(nc, tc, ctx, xt, wcat, out)
    nc.compile()
    return nc


def make_in_maps(x, Wq, Wk, Wv):
    # wcat[p, (m d n)] = W32m.T[d*128+p, n] hi then lo, W32 = 32*W
    whs, wls = [], []
    for W in (Wq, Wk, Wv):
        w32 = np.asarray(W, np.float32).T * 32.0          # [D, n]
        wh = w32.astype(E4)
        wl = (w32 - wh.astype(np.float32)).astype(E4)
        for lst, w8 in ((whs, wh), (wls, wl)):
            wt = w8.reshape(ND, 128, N)                   # [d, p, n]
            lst.append(wt.transpose(1, 0, 2).reshape(128, ND * N))
    wcat = np.ascontiguousarray(
        np.concatenate(whs + wls, axis=1))                # [128, 2*3*ND*N]
    x = np.asarray(x, np.float32)
    in_maps = []
    for c in range(NCORES):
        bb, h = c // 2, c % 2
        xb = x[bb]
        x_part = xb if h == 0 else np.concatenate([xb[SQ:], xb[:SQ]], axis=0)
        # xt[p, (c d two s)] = fp8 hi/lo of x_part[c*512+s, d*128+p]
        xr = x_part.reshape(NCH, CCH, ND, 128).transpose(3, 0, 2, 1)
        xh = xr.astype(E4)                                # [p, c, d, s]
        xl = (xr - xh.astype(np.float32)).astype(E4)
        xt_part = np.ascontiguousarray(
            np.stack([xh, xl], axis=3)                    # [p, c, d, two, s]
            .reshape(128, NCH * XCH))
        in_maps.append({"xt_part": xt_part, "wcat": wcat})
    return in_maps


def kernel(x, Wq, Wk, Wv):
    nc = build_bass()
    res = run_bass_kernel_spmd(nc, make_in_maps(x, Wq, Wk, Wv),
                               core_ids=list(range(NCORES)))
    out = np.empty((B, S, L), dtype=np.float32)
    for c in range(NCORES):
        bb, h = c // 2, c % 2
        # device layout out_dev[l, q]: final rows are columns
        out[bb, h * SQ:(h + 1) * SQ] = res.results[c]["out_part"].T
    return out


# revision 57
# speedup vs baseline: 1.1952x; 1.1952x over previous
"""Single-head attention (B=4, S=4096, D=1024, N=L=128) on 8 trn2 NeuronCores.

Sharding: core c handles batch b = c//2, query half h = c%2 (2048 queries).
Each core receives the full context of its batch with its own query half
ordered FIRST (attention is permutation-invariant over the context axis).

fp8 strategy (per-stage, validated numerically against the f64 reference):
  - Projections run as fp8 DoubleRow matmuls (0.5 cycles/out-col, 2 planes
    of 128 contraction each) in THREE passes: x_hi@W32h (+x_lo plane fused),
    then x_hi@W32l over d-tile pairs. W32 = 32*W is pre-scaled on the host so
    its fp8 encoding avoids the e4m3 subnormal floor (sigma_W = 1/32); the
    32x is folded into the exp scale (q,k) / final epilogue multiply (v).
    Host supplies x as interleaved fp8 (hi, lo) residual pairs, so a proj
    chunk is 8 DR matmuls (pass12: planes = (x_hi_d, x_lo_d) vs duplicated
    W32h_d) + 4 DR matmuls (pass3: planes = d-tile pairs of x_hi vs W32l).
    12*256 cycles vs bf16's 8*512: 25% cheaper at bf16-level accuracy.
  - Scores run as fp8 DoubleRow with stationary [k_hi | k_hi] (stride-0
    plane broadcast) and moving [q_hi | q_lo]: full-precision q times fp8 k
    at 2x bf16 rate. Only the single k quantization (~2.4% rms) enters the
    softmax logits; measured end-to-end rel err ~9e-3 (gate 2e-2).
  - exp on ACT with scale = 1/(sqrt(D)*1024) (q,k both carry 32x).
  - PV stays bf16 (fp8 on either side measured 2-3e-2: over the gate).
Per-engine busy (cost model): PE ~69us, ACT (exp) ~66us, DVE ~55us.

Per-core pipeline (single interleaved emission). The schedule balances PE
against the ACT exp stream (64 x [128,1024] exps ~= 66us, the hard floor):
half 0 runs proj + scores only (PE ~34us ~= ACT), deferring ALL of half 0's
PV matmuls; half 1 runs scores + two PV groups per step (h0's deferred PVs
drain first, then h1's own) so PE stays fed at ~ACT rate throughout. All
half-0 sT tiles stay live in SBUF until their deferred PVs consume them.
PSUM: proj(2 banks)+pss(4) in half 0; pss(4)+po0(2)+po1(2) in half 1 (proj
pool released first); the denominator fold lands in pss-tagged tiles at the
tail. Softmax denominator partials accumulate as bf16 adds on DVE (chains
0-2) / gpsimd (chain 3, SBUF only - gpsimd cannot touch PSUM); chains
pre-fold on DVE then one ones-matmul broadcasts the denominator row across
partitions; epilogue does DVE reciprocal then a single scalar_tensor_tensor
(po * 1/32) * recip before the output DMA.
"""
from contextlib import ExitStack

import numpy as np
import ml_dtypes

import concourse.tile as tile
import concourse.mybir as mybir
from concourse import bacc
from concourse.bass_utils import run_bass_kernel_spmd

B, S, D, N, L = 4, 4096, 1024, 128, 128
NCORES = 8
SQ = B * S // NCORES      # 2048 queries per core
CCH = 512                 # projection chunk (tokens)
NCH = S // CCH            # 8 projection chunks
NKC = S // 128            # 32 kctx subchunks of 128
QH = 1024                 # query half processed per attention sweep
ND = D // 128             # 8 contraction tiles over D
NCHAIN = 4                # denominator partial chains
SCALE = 1.0 / float(np.sqrt(D))
EXP_SCALE = SCALE / 1024.0   # q,k each carry a 32x from W32 host pre-scale

BF = mybir.dt.bfloat16
F32 = mybir.dt.float32
F8 = mybir.dt.float8e4
E4 = ml_dtypes.float8_e4m3
NWARM = 24
UPFRONT_X = 2
Q_ACT_COPY = True
PV0_AT = 14

DR = mybir.MatmulPerfMode.DoubleRow

# xt8 element offsets: [p, (c d two s)], two = (hi, lo)
XCH = ND * 2 * CCH        # elems per chunk per partition (8192)


def emit(nc, tc, ctx, xt, wcat, out):
    UPFRONT_X_ = UPFRONT_X
    persist = ctx.enter_context(tc.tile_pool(name="persist", bufs=1))
    zwarm = persist.tile([128, 128], BF, tag="zwarm")
    nc.gpsimd.memset(zwarm, 0.0)
    ones_mat = persist.tile([128, 128], BF, tag="ones_mat")
    nc.vector.memset(ones_mat, 1.0)

    xbig = persist.tile([128, NCH * XCH], F8, tag="xbig")
    # wcat: W32h [p, (m d n)] then W32l [p, (m d n)], m = (q, k, v)
    WSZ = 3 * ND * N
    wsb = persist.tile([128, 2 * WSZ], F8, tag="wsb")

    # HBM reads serialize at ~343 GB/s, so ALL loads ride one queue (SP) in
    # exactly the order the pipeline consumes them: Wq, x0, Wk, x1 (feeding
    # q0/k0/q1 and the first score group), then Wv and the remaining chunks.
    def xdma(c, parts=1):
        w = XCH // parts
        for s in range(parts):
            sl = slice(c * XCH + s * w, c * XCH + (s + 1) * w)
            nc.sync.dma_start(out=xbig[:, sl], in_=xt[:, sl])

    W1 = 2 * ND * N   # per-matrix (hi | lo) block
    half = XCH // 2

    def wdma(m):
        nc.sync.dma_start(out=wsb[:, m * W1:(m + 1) * W1],
                          in_=wcat[:, m * W1:(m + 1) * W1])

    # x6/x7 are issued lazily from the piece weave so the early vv XBAR
    # transposes (same SP queue) interleave with them instead of queuing
    # behind the whole x stream.
    wdma(1)
    xdma(0, parts=4)
    wdma(0)
    xdma(1, parts=2)
    wdma(2)
    for c in range(2, 2 + UPFRONT_X):
        xdma(c)

    def wh2(m, d):
        # stationary [128, 2, 128]: duplicated W32h_d planes (stride-0)
        off = m * W1 + d * N
        return wsb[:, off:off + N].unsqueeze(1).broadcast_to((128, 2, N))

    def wl2(m, t):
        # stationary [128, 2, 128]: planes (W32l_{2t}, W32l_{2t+1})
        off = m * W1 + ND * N + 2 * t * N
        return wsb[:, off:off + 2 * N].rearrange("p (two n) -> p two n",
                                                 two=2)

    def x12(c, d):
        # moving [128, 2, 512]: planes (x_hi_d, x_lo_d), contiguous
        off = c * XCH + d * 2 * CCH
        return xbig[:, off:off + 2 * CCH].rearrange(
            "p (two s) -> p two s", two=2)

    def x3(c, t):
        # moving [128, 2, 512]: planes (x_hi_{2t}, x_hi_{2t+1}), d-stride 1024
        off = c * XCH + 2 * t * 2 * CCH
        return xbig[:, off:off + 3 * CCH].rearrange(
            "p (d s) -> p d s", d=3)[:, 0::2, :]

    def proj_mm_a(ps, m, c):
        for d in range(ND):
            nc.tensor.matmul(ps, wh2(m, d), x12(c, d),
                             start=(d == 0), stop=False, perf_mode=DR)

    def proj_mm_b(ps, m, c):
        for t in range(ND // 2):
            nc.tensor.matmul(ps, wl2(m, t), x3(c, t),
                             start=False, stop=(t == ND // 2 - 1),
                             perf_mode=DR)

    kT8 = persist.tile([128, S], F8, tag="kT8")     # [n, kctx] fp8 (32x)
    vv = persist.tile([128, S], BF, tag="vv")       # 32 chunks [kctx128, l]
    qhl = persist.tile([128, 2 * SQ], F8, tag="qhl")  # per 512-q: (hi, lo)

    vtc_pool = ctx.enter_context(tc.tile_pool(name="vtc", bufs=3))

    # ---------------- attention pools (outer; proj pool nests inside) ----
    spool = ctx.enter_context(tc.tile_pool(name="sT", bufs=1))
    parts = ctx.enter_context(tc.tile_pool(name="parts", bufs=1))
    pss_pool = ctx.enter_context(tc.tile_pool(name="pss", bufs=2, space="PSUM"))
    po_pool = ctx.enter_context(tc.tile_pool(name="po", bufs=1, space="PSUM"))
    epi_sb = ctx.enter_context(tc.tile_pool(name="episb", bufs=2))

    proj_cm = tc.tile_pool(name="proj", bufs=2, space="PSUM")
    proj_ps = proj_cm.__enter__()
    po1_cm = tc.tile_pool(name="po1", bufs=1, space="PSUM")
    po1_holder = {"pool": None}

    # PE warmup while the first DMAs land (p-state ramp off critical path);
    # sized to end as x0 arrives so proj starts immediately at full rate
    for _ in range(NWARM):
        pwarm = proj_ps.tile([128, CCH], F32, tag="proj", name="pwarm")
        nc.tensor.matmul(pwarm[:, 0:128], zwarm, zwarm, start=True,
                         stop=True)

    vt_pending = []

    def flush_vt(n=None):
        while vt_pending and (n is None or len(vt_pending) > n):
            csl, vTc = vt_pending.pop(0)
            nc.sync.dma_start_transpose(
                out=vv[:, csl].rearrange("p (t q) -> p t q", t=CCH // 128),
                in_=vTc)

    # Each proj piece is emitted as two parts (pass12 / pass3+copies) so a
    # pending score group never waits behind a full 12-matmul chain.
    def piece_k(c):
        cell = {}
        csl = slice(c * CCH, (c + 1) * CCH)

        def a():
            if 2 + UPFRONT_X <= c + 2 < NCH:
                xdma(c + 2)
            cell["ps"] = proj_ps.tile([128, CCH], F32, tag="proj", name="pk")
            proj_mm_a(cell["ps"], 1, c)

        def b():
            proj_mm_b(cell["ps"], 1, c)
            # split the quantize so the first kctx subchunks unlock early
            h = CCH // 2
            nc.vector.tensor_copy(kT8[:, csl][:, 0:h], cell["ps"][:, 0:h])
            nc.vector.tensor_copy(kT8[:, csl][:, h:], cell["ps"][:, h:])
        return [a, b]

    def piece_v(c):
        cell = {}
        csl = slice(c * CCH, (c + 1) * CCH)

        def a():
            cell["ps"] = proj_ps.tile([128, CCH], F32, tag="proj", name="pv")
            proj_mm_a(cell["ps"], 2, c)

        def b():
            proj_mm_b(cell["ps"], 2, c)
            vTc = vtc_pool.tile([128, CCH], BF, tag="vTc", name=f"vTc{c % 3}")
            nc.vector.tensor_copy(vTc, cell["ps"])
            # defer the XBAR transpose issue one piece so it queues behind
            # the next x chunk's transfer on the SP DMA queue
            vt_pending.append((csl, vTc))
            flush_vt(1)
        return [a, b]

    def piece_q(c):
        cell = {}
        hi = slice(c * 2 * CCH, c * 2 * CCH + CCH)
        lo = slice(c * 2 * CCH + CCH, (c + 1) * 2 * CCH)

        def a():
            cell["ps"] = proj_ps.tile([128, CCH], F32, tag="proj", name="pq")
            proj_mm_a(cell["ps"], 0, c)

        def b():
            proj_mm_b(cell["ps"], 0, c)
            if c < 2 and Q_ACT_COPY:
                nc.scalar.activation(qhl[:, hi], cell["ps"],
                                     func=mybir.ActivationFunctionType.Copy)
            else:
                nc.vector.tensor_copy(qhl[:, hi], cell["ps"])
            nc.vector.tensor_tensor(out=qhl[:, lo], in0=cell["ps"],
                                    in1=qhl[:, hi],
                                    op=mybir.AluOpType.subtract)
        return [a, b]

    po = [None, None]
    part = {}
    sTs = {}
    NH0 = 8   # PV groups executed within half 0 (rest defer into half 1)
    LAG = 3   # PV trails its score by LAG steps (exp-gated pacing)

    def score_mm(h, i, j, out):
        ksl = slice(i * 128, (i + 1) * 128)
        kst = kT8[:, ksl].unsqueeze(1).broadcast_to((128, 2, 128))
        blk = h * 2 + j
        qmv = qhl[:, blk * 2 * CCH:(blk + 1) * 2 * CCH].rearrange(
            "p (two s) -> p two s", two=2)
        nc.tensor.matmul(out, kst, qmv, start=True, stop=True,
                         perf_mode=DR, skip_group_check=True)

    def sT_tag(h, i):
        # every h0 sT gets a dedicated buffer (PVs may run much later than
        # the score);  h1 rotates 8 bufs (PV trails by <= 8 there)
        return f"sTd{i}" if h == 0 else f"sTb{i % 8}"

    s512_ps = {}

    def emit_score512(h, i, j):
        # startup path: one j-block at a time (j0 needs only k0/q0); pss
        # tiles are shared by i-pairs so the pool rotation stays 2-deep
        key = (i // 2, j)
        if key not in s512_ps:
            s512_ps[key] = pss_pool.tile([128, QH], F32, tag="pss",
                                         name=f"pss512_{key}")
        ps = s512_ps[key][:, (i % 2) * CCH:(i % 2 + 1) * CCH]
        score_mm(h, i, j, ps)
        if j == 0:
            tg = sT_tag(h, i)
            sTs[h, i] = spool.tile([128, QH], BF, tag=tg, name=tg)
        sT = sTs[h, i]
        jsl = slice(j * (QH // 2), (j + 1) * (QH // 2))
        nc.scalar.activation(sT[:, jsl], ps,
                             func=mybir.ActivationFunctionType.Exp,
                             scale=EXP_SCALE)
        sTs[h, i] = sT
        if j == 1:
            emit_chain(h, i, sT)

    def emit_attn_score(h, i, split_exp=False):
        pss = pss_pool.tile([128, QH], F32, tag="pss")
        for j in range(2):
            score_mm(h, i, j,
                     pss[:, j * (QH // 2):(j + 1) * (QH // 2)])
        tg = sT_tag(h, i)
        sT = spool.tile([128, QH], BF, tag=tg, name=tg)
        sTs[h, i] = sT
        if split_exp:
            # last group of a half: per-j exps so the j0 denominator fold
            # starts one half-exp earlier
            for j in range(2):
                jsl = slice(j * (QH // 2), (j + 1) * (QH // 2))
                nc.scalar.activation(sT[:, jsl], pss[:, jsl],
                                     func=mybir.ActivationFunctionType.Exp,
                                     scale=EXP_SCALE)
                nc.vector.tensor_add(part[h, 3][:, jsl],
                                     part[h, 3][:, jsl], sT[:, jsl])
            return
        nc.scalar.activation(sT, pss, func=mybir.ActivationFunctionType.Exp,
                             scale=EXP_SCALE)
        emit_chain(h, i, sT)

    def emit_chain(h, i, sT):
        # chains 2,3 ride gpsimd (SBUF-only engine, otherwise idle) except
        # near the end of a half so the fold is never gated on its slow queue
        ch = i % NCHAIN
        eng = nc.gpsimd if (ch >= 2 and i < NKC - 8) else nc.vector
        if i < NCHAIN:
            part[h, ch] = parts.tile([128, QH], BF, tag=f"part{h}_{ch}",
                                     name=f"part{h}_{ch}")
            eng.tensor_copy(part[h, ch], sT)
        elif i == NKC - 1:
            # column-split the final add so the j0 fold/reciprocal chain can
            # start half an add earlier
            for j in range(2):
                jsl = slice(j * (QH // 2), (j + 1) * (QH // 2))
                eng.tensor_add(part[h, ch][:, jsl], part[h, ch][:, jsl],
                               sT[:, jsl])
        else:
            eng.tensor_add(part[h, ch], part[h, ch], sT)

    def emit_attn_pv(h, i):
        ksl = slice(i * 128, (i + 1) * 128)
        if i == 0:
            # po0 shares half-0's psum budget with proj; po1's pool is
            # entered only after the proj pool releases its banks
            pool = po_pool if h == 0 else po1_holder["pool"]
            po[h] = pool.tile([128, QH], F32, tag=f"po{h}", name=f"po{h}")
        for j in range(2):
            jsl = slice(j * (QH // 2), (j + 1) * (QH // 2))
            nc.tensor.matmul(po[h][:, jsl], vv[:, ksl], sTs[h, i][:, jsl],
                             start=(i == 0), stop=(i == NKC - 1),
                             skip_group_check=True)

    prsR = [None, None]

    def emit_fold_pre12(h):
        # chains 1,2 close at steps NKC-3/NKC-2, so they can collapse into
        # chain 0 while the final exp still runs
        for ch in (1, 2):
            nc.vector.tensor_add(part[h, 0], part[h, 0], part[h, ch])

    def emit_fold_pre3(h, j):
        jsl = slice(j * (QH // 2), (j + 1) * (QH // 2))
        nc.vector.tensor_add(part[h, 0][:, jsl], part[h, 0][:, jsl],
                             part[h, 3][:, jsl])

    def emit_fold_mm(h, j):
        # ones-matmul broadcasts the denominator row to every partition;
        # output lands in a pss-tagged tile (free at the tail)
        if j == 0:
            prsR[h] = pss_pool.tile([128, QH], F32, tag="pss",
                                    name=f"prsR{h}")
        jsl = slice(j * (QH // 2), (j + 1) * (QH // 2))
        nc.tensor.matmul(prsR[h][:, jsl], ones_mat, part[h, 0][:, jsl],
                         start=True, stop=True, skip_group_check=True)

    def emit_epi_finish(h):
        for j in range(2):
            jsl = slice(j * (QH // 2), (j + 1) * (QH // 2))
            recipB = epi_sb.tile([128, QH // 2], F32, tag="recipB")
            nc.vector.reciprocal(recipB, prsR[h][:, jsl])
            foall = epi_sb.tile([128, QH // 2], F32, tag="foall",
                                name=f"foall{j}")
            # v carries a 32x from W32v: out = (po * 1/32) * recip; quarter
            # columns pipeline the multiply with the output DMA
            for q in range(2):
                qsl = slice(q * (QH // 4), (q + 1) * (QH // 4))
                base = h * QH + j * (QH // 2) + q * (QH // 4)
                nc.vector.scalar_tensor_tensor(
                    out=foall[:, qsl], in0=po[h][:, jsl][:, qsl],
                    scalar=1.0 / 32.0, in1=recipB[:, qsl],
                    op0=mybir.AluOpType.mult, op1=mybir.AluOpType.mult)
                nc.sync.dma_start(out=out[:, base:base + QH // 4],
                                  in_=foall[:, qsl])

    # ---------------- schedule ----------------
    # Half 0: proj piece-parts woven into the score steps (q0,k0,q1 up front
    # so the first exp lands early; k(c) hard-gated before step 4c; q2/q3
    # late). Only PV groups 0..NH0-1 run here (LAG-gated) — h0 stays just
    # under the ACT exp rate. The remaining PV groups defer into half 1 as a
    # ready reservoir the scheduler can pull into any PE idle slot.
    k_done = {"n": 0}

    def mk_k(c):
        a, b = piece_k(c)

        def b2():
            b()
            k_done["n"] = c + 1
        return [a, b2]

    rest = (mk_k(1) + piece_v(0) + piece_v(1) + mk_k(2) + piece_v(2)
            + mk_k(3) + piece_v(3) + mk_k(4) + piece_v(4) + mk_k(5)
            + piece_q(2) + piece_v(5) + mk_k(6) + piece_q(3) + piece_v(6)
            + mk_k(7) + piece_v(7))

    # Startup: k0/q0 feed four 512-wide j0 score groups (their exps start
    # while x1 is still in flight), then q1 unlocks the j1 halves.
    for f in mk_k(0) + piece_q(0):
        f()
    for i in range(4):
        emit_score512(0, i, 0)
    for f in piece_q(1):
        f()
    for i in range(4):
        emit_score512(0, i, 1)

    emitted = 0
    for i in range(4, NKC):
        need_k = i // 4 + 1
        while emitted < len(rest) and k_done["n"] < need_k:
            rest[emitted]()
            emitted += 1
        emit_attn_score(0, i)
        if PV0_AT <= i < NH0 + PV0_AT:
            emit_attn_pv(0, i - PV0_AT)
        while emitted < len(rest) and emitted * 26 < len(rest) * (i - 3):
            rest[emitted]()
            emitted += 1
    while emitted < len(rest):
        rest[emitted]()
        emitted += 1
    flush_vt(0)
    proj_cm.__exit__(None, None, None)
    po1_holder["pool"] = po1_cm.__enter__()

    # Half 1: scores with own PVs LAG-gated plus one deferred-h0 PV per
    # step (reservoir drains by step ~23); h0's fold/epilogue runs mid-
    # stream once po0 closes, so only h1's epilogue remains in the tail.
    emit_fold_pre12(0)
    emit_fold_pre3(0, 0)
    emit_fold_pre3(0, 1)
    pv0 = list(range(NH0, NKC))
    for s in range(NKC):
        emit_attn_score(1, s, split_exp=(s == NKC - 1))
        if s >= LAG:
            emit_attn_pv(1, s - LAG)
        if pv0:
            emit_attn_pv(0, pv0.pop(0))
        if s == 26:
            emit_fold_mm(0, 0)
            emit_fold_mm(0, 1)
            emit_epi_finish(0)
        elif s == NKC - 3:
            # chain 1 closed at step NKC-3: collapse into chain 0 now
            nc.vector.tensor_add(part[1, 0], part[1, 0], part[1, 1])
        elif s == NKC - 2:
            nc.vector.tensor_add(part[1, 0], part[1, 0], part[1, 2])
    for s in range(NKC - LAG, NKC):
        emit_attn_pv(1, s)
    for j in range(2):
        emit_fold_pre3(1, j)
        emit_fold_mm(1, j)
    emit_epi_finish(1)


def build_bass(iters=1):
    nc = bacc.Bacc()
    xt = nc.dram_tensor("xt_part", [128, NCH * XCH], F8,
                        kind="ExternalInput")
    wcat = nc.dram_tensor("wcat", [128, 2 * 3 * ND * N], F8,
                          kind="ExternalInput")
    out = nc.dram_tensor("out_part", [128, (SQ // 128) * L], F32,
                         kind="ExternalOutput")
    with tile.TileContext(nc) as tc:
        for _ in range(iters):
            with ExitStack() as ctx:
                emit(nc, tc, ctx, xt, wcat, out)
    nc.compile()
    return nc


def make_in_maps(x, Wq, Wk, Wv):
    # wcat[p, (m two d n)] = W32m.T[d*128+p, n], two = (hi, lo), W32 = 32*W
    blocks = []
    for W in (Wq, Wk, Wv):
        w32 = np.asarray(W, np.float32).T * 32.0          # [D, n]
        wh = w32.astype(E4)
        wl = (w32 - wh.astype(np.float32)).astype(E4)
        for w8 in (wh, wl):
            wt = w8.reshape(ND, 128, N)                   # [d, p, n]
            blocks.append(wt.transpose(1, 0, 2).reshape(128, ND * N))
    wcat = np.ascontiguousarray(
        np.concatenate(blocks, axis=1))                   # [128, 2*3*ND*N]
    x = np.asarray(x, np.float32)
    in_maps = []
    for c in range(NCORES):
        bb, h = c // 2, c % 2
        xb = x[bb]
        x_part = xb if h == 0 else np.concatenate([xb[SQ:], xb[:SQ]], axis=0)
        # xt[p, (c d two s)] = fp8 hi/lo of x_part[c*512+s, d*128+p]
        xr = x_part.reshape(NCH, CCH, ND, 128).transpose(3, 0, 2, 1)
        xh = xr.astype(E4)                                # [p, c, d, s]
        xl = (xr - xh.astype(np.float32)).astype(E4)
        xt_part = np.ascontiguousarray(
            np.stack([xh, xl], axis=3)                    # [p, c, d, two, s]
            .reshape(128, NCH * XCH))
        in_maps.append({"xt_part": xt_part, "wcat": wcat})
    return in_maps


def kernel(x, Wq, Wk, Wv):
    nc = build_bass()
    res = run_bass_kernel_spmd(nc, make_in_maps(x, Wq, Wk, Wv),
                               core_ids=list(range(NCORES)))
    out = np.empty((B, S, L), dtype=np.float32)
    for c in range(NCORES):
        bb, h = c // 2, c % 2
        # device layout out_dev[l, q]: final rows are columns
        out[bb, h * SQ:(h + 1) * SQ] = res.results[c]["out_part"].T
    return out


# revision 65
# speedup vs baseline: 1.2350x; 1.0333x over previous
"""Single-head attention (B=4, S=4096, D=1024, N=L=128) on 8 trn2 NeuronCores.

Sharding: core c handles batch b = c//2, query half h = c%2 (2048 queries).
Each core receives the full context of its batch with its own query half
ordered FIRST (attention is permutation-invariant over the context axis).

fp8 strategy (per-stage, validated numerically against the f64 reference):
  - Projections run as fp8 DoubleRow matmuls (0.5 cycles/out-col, 2 planes
    of 128 contraction each) in THREE passes: x_hi@W32h (+x_lo plane fused),
    then x_hi@W32l over d-tile pairs. W32 = 32*W is pre-scaled on the host so
    its fp8 encoding avoids the e4m3 subnormal floor (sigma_W = 1/32); the
    32x is folded into the exp scale (q,k) / final epilogue multiply (v).
    Host supplies x as interleaved fp8 (hi, lo) residual pairs, so a proj
    chunk is 8 DR matmuls (pass12: planes = (x_hi_d, x_lo_d) vs duplicated
    W32h_d) + 4 DR matmuls (pass3: planes = d-tile pairs of x_hi vs W32l).
    12*256 cycles vs bf16's 8*512: 25% cheaper at bf16-level accuracy.
  - Scores run as fp8 DoubleRow with stationary [k_hi | k_hi] (stride-0
    plane broadcast) and moving [q_hi | q_lo]: full-precision q times fp8 k
    at 2x bf16 rate. Only the single k quantization (~2.4% rms) enters the
    softmax logits; measured end-to-end rel err ~9e-3 (gate 2e-2).
  - exp on ACT with scale = 1/(sqrt(D)*1024) (q,k both carry 32x).
  - PV stays bf16 (fp8 on either side measured 2-3e-2: over the gate).
Per-engine busy (cost model): PE ~69us, ACT (exp) ~66us, DVE ~55us.

Per-core pipeline (single interleaved emission). The schedule balances PE
against the ACT exp stream (64 x [128,1024] exps ~= 66us, the hard floor):
half 0 runs proj + scores only (PE ~34us ~= ACT), deferring ALL of half 0's
PV matmuls; half 1 runs scores + two PV groups per step (h0's deferred PVs
drain first, then h1's own) so PE stays fed at ~ACT rate throughout. All
half-0 sT tiles stay live in SBUF until their deferred PVs consume them.
PSUM: proj(2 banks)+pss(4) in half 0; pss(4)+po0(2)+po1(2) in half 1 (proj
pool released first); the denominator fold lands in pss-tagged tiles at the
tail. Softmax denominator partials accumulate as bf16 adds on DVE (chains
0-2) / gpsimd (chain 3, SBUF only - gpsimd cannot touch PSUM); chains
pre-fold on DVE then one ones-matmul broadcasts the denominator row across
partitions; epilogue does DVE reciprocal then a single scalar_tensor_tensor
(po * 1/32) * recip before the output DMA.
"""
from contextlib import ExitStack

import numpy as np
import ml_dtypes

import concourse.tile as tile
import concourse.mybir as mybir
from concourse import bacc
from concourse.bass_utils import run_bass_kernel_spmd

B, S, D, N, L = 4, 4096, 1024, 128, 128
NCORES = 8
SQ = B * S // NCORES      # 2048 queries per core
CCH = 512                 # projection chunk (tokens)
NCH = S // CCH            # 8 projection chunks
NKC = S // 128            # 32 kctx subchunks of 128
QH = 1024                 # query half processed per attention sweep
ND = D // 128             # 8 contraction tiles over D
NCHAIN = 4                # denominator partial chains
SCALE = 1.0 / float(np.sqrt(D))
EXP_SCALE = SCALE / 1024.0   # q,k each carry a 32x from W32 host pre-scale

BF = mybir.dt.bfloat16
F32 = mybir.dt.float32
F8 = mybir.dt.float8e4
E4 = ml_dtypes.float8_e4m3
NWARM = 24
UPFRONT_X = 4
Q_ACT_COPY = True
PV0_AT = 14
POPS = 1
FOLD0_AT = 26

DR = mybir.MatmulPerfMode.DoubleRow

# xt8 element offsets: [p, (c d two s)], two = (hi, lo)
XCH = ND * 2 * CCH        # elems per chunk per partition (8192)


def emit(nc, tc, ctx, xt, wcat, out):
    UPFRONT_X_ = UPFRONT_X
    persist = ctx.enter_context(tc.tile_pool(name="persist", bufs=1))
    zwarm = persist.tile([128, 128], BF, tag="zwarm")
    nc.gpsimd.memset(zwarm, 0.0)
    ones_mat = persist.tile([128, 128], BF, tag="ones_mat")
    nc.vector.memset(ones_mat, 1.0)

    xbig = persist.tile([128, NCH * XCH], F8, tag="xbig")
    # wcat: W32h [p, (m d n)] then W32l [p, (m d n)], m = (q, k, v)
    WSZ = 3 * ND * N
    wsb = persist.tile([128, 2 * WSZ], F8, tag="wsb")

    # HBM reads serialize at ~343 GB/s, so ALL loads ride one queue (SP) in
    # exactly the order the pipeline consumes them: Wq, x0, Wk, x1 (feeding
    # q0/k0/q1 and the first score group), then Wv and the remaining chunks.
    def xdma(c, parts=1):
        w = XCH // parts
        for s in range(parts):
            sl = slice(c * XCH + s * w, c * XCH + (s + 1) * w)
            nc.sync.dma_start(out=xbig[:, sl], in_=xt[:, sl])

    W1 = 2 * ND * N   # per-matrix (hi | lo) block
    half = XCH // 2

    def wdma(m):
        nc.sync.dma_start(out=wsb[:, m * W1:(m + 1) * W1],
                          in_=wcat[:, m * W1:(m + 1) * W1])

    # x6/x7 are issued lazily from the piece weave so the early vv XBAR
    # transposes (same SP queue) interleave with them instead of queuing
    # behind the whole x stream.
    wdma(1)
    xdma(0, parts=4)
    wdma(0)
    xdma(1, parts=2)
    wdma(2)
    for c in range(2, 2 + UPFRONT_X):
        xdma(c)

    def wh2(m, d):
        # stationary [128, 2, 128]: duplicated W32h_d planes (stride-0)
        off = m * W1 + d * N
        return wsb[:, off:off + N].unsqueeze(1).broadcast_to((128, 2, N))

    def wl2(m, t):
        # stationary [128, 2, 128]: planes (W32l_{2t}, W32l_{2t+1})
        off = m * W1 + ND * N + 2 * t * N
        return wsb[:, off:off + 2 * N].rearrange("p (two n) -> p two n",
                                                 two=2)

    def x12(c, d):
        # moving [128, 2, 512]: planes (x_hi_d, x_lo_d), contiguous
        off = c * XCH + d * 2 * CCH
        return xbig[:, off:off + 2 * CCH].rearrange(
            "p (two s) -> p two s", two=2)

    def x3(c, t):
        # moving [128, 2, 512]: planes (x_hi_{2t}, x_hi_{2t+1}), d-stride 1024
        off = c * XCH + 2 * t * 2 * CCH
        return xbig[:, off:off + 3 * CCH].rearrange(
            "p (d s) -> p d s", d=3)[:, 0::2, :]

    def proj_mm_a(ps, m, c):
        for d in range(ND):
            nc.tensor.matmul(ps, wh2(m, d), x12(c, d),
                             start=(d == 0), stop=False, perf_mode=DR)

    def proj_mm_b(ps, m, c):
        for t in range(ND // 2):
            nc.tensor.matmul(ps, wl2(m, t), x3(c, t),
                             start=False, stop=(t == ND // 2 - 1),
                             perf_mode=DR)

    kT8 = persist.tile([128, S], F8, tag="kT8")     # [n, kctx] fp8 (32x)
    vv = persist.tile([128, S], BF, tag="vv")       # 32 chunks [kctx128, l]
    qhl = persist.tile([128, 2 * SQ], F8, tag="qhl")  # per 512-q: (hi, lo)

    vtc_pool = ctx.enter_context(tc.tile_pool(name="vtc", bufs=3))

    # ---------------- attention pools (outer; proj pool nests inside) ----
    spool = ctx.enter_context(tc.tile_pool(name="sT", bufs=1))
    parts = ctx.enter_context(tc.tile_pool(name="parts", bufs=1))
    pss_pool = ctx.enter_context(tc.tile_pool(name="pss", bufs=2, space="PSUM"))
    po_pool = ctx.enter_context(tc.tile_pool(name="po", bufs=1, space="PSUM"))
    epi_sb = ctx.enter_context(tc.tile_pool(name="episb", bufs=2))

    proj_cm = tc.tile_pool(name="proj", bufs=2, space="PSUM")
    proj_ps = proj_cm.__enter__()
    po1_cm = tc.tile_pool(name="po1", bufs=1, space="PSUM")
    po1_holder = {"pool": None}

    # PE warmup while the first DMAs land (p-state ramp off critical path);
    # sized to end as x0 arrives so proj starts immediately at full rate
    for _ in range(NWARM):
        pwarm = proj_ps.tile([128, CCH], F32, tag="proj", name="pwarm")
        nc.tensor.matmul(pwarm[:, 0:128], zwarm, zwarm, start=True,
                         stop=True)

    vt_pending = []

    def flush_vt(n=None):
        while vt_pending and (n is None or len(vt_pending) > n):
            csl, vTc = vt_pending.pop(0)
            nc.sync.dma_start_transpose(
                out=vv[:, csl].rearrange("p (t q) -> p t q", t=CCH // 128),
                in_=vTc)

    # Each proj piece is emitted as two parts (pass12 / pass3+copies) so a
    # pending score group never waits behind a full 12-matmul chain.
    def piece_k(c):
        cell = {}
        csl = slice(c * CCH, (c + 1) * CCH)

        def a():
            if 2 + UPFRONT_X <= c + 2 < NCH:
                xdma(c + 2)
            cell["ps"] = proj_ps.tile([128, CCH], F32, tag="proj", name="pk")
            proj_mm_a(cell["ps"], 1, c)

        def b():
            proj_mm_b(cell["ps"], 1, c)
            # split the quantize so the first kctx subchunks unlock early
            h = CCH // 2
            nc.vector.tensor_copy(kT8[:, csl][:, 0:h], cell["ps"][:, 0:h])
            nc.vector.tensor_copy(kT8[:, csl][:, h:], cell["ps"][:, h:])
        return [a, b]

    def piece_v(c):
        cell = {}
        csl = slice(c * CCH, (c + 1) * CCH)

        def a():
            cell["ps"] = proj_ps.tile([128, CCH], F32, tag="proj", name="pv")
            proj_mm_a(cell["ps"], 2, c)

        def b():
            proj_mm_b(cell["ps"], 2, c)
            vTc = vtc_pool.tile([128, CCH], BF, tag="vTc", name=f"vTc{c % 3}")
            nc.vector.tensor_copy(vTc, cell["ps"])
            # defer the XBAR transpose issue one piece so it queues behind
            # the next x chunk's transfer on the SP DMA queue
            vt_pending.append((csl, vTc))
            flush_vt(1)
        return [a, b]

    def piece_q(c):
        cell = {}
        hi = slice(c * 2 * CCH, c * 2 * CCH + CCH)
        lo = slice(c * 2 * CCH + CCH, (c + 1) * 2 * CCH)

        def a():
            cell["ps"] = proj_ps.tile([128, CCH], F32, tag="proj", name="pq")
            proj_mm_a(cell["ps"], 0, c)

        def b():
            proj_mm_b(cell["ps"], 0, c)
            if c < 2 and Q_ACT_COPY:
                nc.scalar.activation(qhl[:, hi], cell["ps"],
                                     func=mybir.ActivationFunctionType.Copy)
            else:
                nc.vector.tensor_copy(qhl[:, hi], cell["ps"])
            nc.vector.tensor_tensor(out=qhl[:, lo], in0=cell["ps"],
                                    in1=qhl[:, hi],
                                    op=mybir.AluOpType.subtract)
        return [a, b]

    po = [None, None]
    part = {}
    sTs = {}
    NH0 = 8   # PV groups executed within half 0 (rest defer into half 1)
    LAG = 3   # PV trails its score by LAG steps (exp-gated pacing)

    def score_mm(h, i, j, out):
        ksl = slice(i * 128, (i + 1) * 128)
        kst = kT8[:, ksl].unsqueeze(1).broadcast_to((128, 2, 128))
        blk = h * 2 + j
        qmv = qhl[:, blk * 2 * CCH:(blk + 1) * 2 * CCH].rearrange(
            "p (two s) -> p two s", two=2)
        nc.tensor.matmul(out, kst, qmv, start=True, stop=True,
                         perf_mode=DR, skip_group_check=True)

    def sT_tag(h, i):
        # every h0 sT gets a dedicated buffer (PVs may run much later than
        # the score);  h1 rotates 8 bufs (PV trails by <= 8 there)
        return f"sTd{i}" if h == 0 else f"sTb{i % 8}"

    s512_ps = {}

    def emit_score512(h, i, j):
        # startup path: one j-block at a time (j0 needs only k0/q0); pss
        # tiles are shared by i-pairs so the pool rotation stays 2-deep
        key = (i // 2, j)
        if key not in s512_ps:
            s512_ps[key] = pss_pool.tile([128, QH], F32, tag="pss",
                                         name=f"pss512_{key}")
        ps = s512_ps[key][:, (i % 2) * CCH:(i % 2 + 1) * CCH]
        score_mm(h, i, j, ps)
        if j == 0:
            tg = sT_tag(h, i)
            sTs[h, i] = spool.tile([128, QH], BF, tag=tg, name=tg)
        sT = sTs[h, i]
        jsl = slice(j * (QH // 2), (j + 1) * (QH // 2))
        nc.scalar.activation(sT[:, jsl], ps,
                             func=mybir.ActivationFunctionType.Exp,
                             scale=EXP_SCALE)
        sTs[h, i] = sT
        if j == 1:
            emit_chain(h, i, sT)

    def emit_attn_score(h, i, split_exp=False):
        pss = pss_pool.tile([128, QH], F32, tag="pss")
        for j in range(2):
            score_mm(h, i, j,
                     pss[:, j * (QH // 2):(j + 1) * (QH // 2)])
        tg = sT_tag(h, i)
        sT = spool.tile([128, QH], BF, tag=tg, name=tg)
        sTs[h, i] = sT
        if split_exp:
            # last group of a half: per-j exps, each immediately closing its
            # denominator-fold psum group so recip(j0) starts one half-exp
            # early with only a single matmul in between
            for j in range(2):
                jsl = slice(j * (QH // 2), (j + 1) * (QH // 2))
                nc.scalar.activation(sT[:, jsl], pss[:, jsl],
                                     func=mybir.ActivationFunctionType.Exp,
                                     scale=EXP_SCALE)
                emit_fold_mms(h, j, [sT], start=False, stop=True)
            return
        nc.scalar.activation(sT, pss, func=mybir.ActivationFunctionType.Exp,
                             scale=EXP_SCALE)
        emit_chain(h, i, sT)

    def emit_chain(h, i, sT):
        # chains 2,3 ride gpsimd (SBUF-only engine, otherwise idle) except
        # near the end of a half so the fold is never gated on its slow
        # queue. The last 4 steps skip the vector add entirely: their sT
        # tiles feed the denominator fold matmuls directly.
        if i >= NKC - NCHAIN:
            return
        ch = i % NCHAIN
        eng = nc.gpsimd if (ch >= 2 and i < NKC - 8) else nc.vector
        if i < NCHAIN:
            part[h, ch] = parts.tile([128, QH], BF, tag=f"part{h}_{ch}",
                                     name=f"part{h}_{ch}")
            eng.tensor_copy(part[h, ch], sT)
        else:
            eng.tensor_add(part[h, ch], part[h, ch], sT)

    def emit_attn_pv(h, i):
        ksl = slice(i * 128, (i + 1) * 128)
        if i == 0:
            # po0 shares half-0's psum budget with proj; po1's pool is
            # entered only after the proj pool releases its banks
            pool = po_pool if h == 0 else po1_holder["pool"]
            po[h] = pool.tile([128, QH], F32, tag=f"po{h}", name=f"po{h}")
        for j in range(2):
            jsl = slice(j * (QH // 2), (j + 1) * (QH // 2))
            nc.tensor.matmul(po[h][:, jsl], vv[:, ksl], sTs[h, i][:, jsl],
                             start=(i == 0), stop=(i == NKC - 1),
                             skip_group_check=True)

    prsR = [None, None]

    # The denominator fold is pure PE work: ones-matmuls accumulate the four
    # chain partials plus the last four sT tiles directly into prsR (every
    # partition gets the full denominator row), so no vector adds sit between
    # the final exp and the reciprocal.
    def fold_terms(h):
        return [part[h, ch] for ch in range(NCHAIN)]

    def emit_fold_mms(h, j, terms, start, stop):
        jsl = slice(j * (QH // 2), (j + 1) * (QH // 2))
        for t, term in enumerate(terms):
            nc.tensor.matmul(prsR[h][:, jsl], ones_mat, term[:, jsl],
                             start=(start and t == 0),
                             stop=(stop and t == len(terms) - 1),
                             skip_group_check=True)

    def emit_epi_finish(h):
        for j in range(2):
            jsl = slice(j * (QH // 2), (j + 1) * (QH // 2))
            recipB = epi_sb.tile([128, QH // 2], F32, tag="recipB")
            nc.vector.reciprocal(recipB, prsR[h][:, jsl])
            foall = epi_sb.tile([128, QH // 2], F32, tag="foall",
                                name=f"foall{j}")
            # v carries a 32x from W32v: out = (po * 1/32) * recip; quarter
            # columns pipeline the multiply with the output DMA
            for q in range(2):
                qsl = slice(q * (QH // 4), (q + 1) * (QH // 4))
                base = h * QH + j * (QH // 2) + q * (QH // 4)
                nc.vector.scalar_tensor_tensor(
                    out=foall[:, qsl], in0=po[h][:, jsl][:, qsl],
                    scalar=1.0 / 32.0, in1=recipB[:, qsl],
                    op0=mybir.AluOpType.mult, op1=mybir.AluOpType.mult)
                nc.sync.dma_start(out=out[:, base:base + QH // 4],
                                  in_=foall[:, qsl])

    # ---------------- schedule ----------------
    # Half 0: proj piece-parts woven into the score steps (q0,k0,q1 up front
    # so the first exp lands early; k(c) hard-gated before step 4c; q2/q3
    # late). Only PV groups 0..NH0-1 run here (LAG-gated) — h0 stays just
    # under the ACT exp rate. The remaining PV groups defer into half 1 as a
    # ready reservoir the scheduler can pull into any PE idle slot.
    k_done = {"n": 0}

    def mk_k(c):
        a, b = piece_k(c)

        def b2():
            b()
            k_done["n"] = c + 1
        return [a, b2]

    rest = (mk_k(1) + piece_v(0) + piece_v(1) + mk_k(2) + piece_v(2)
            + mk_k(3) + piece_v(3) + mk_k(4) + piece_v(4) + mk_k(5)
            + piece_q(2) + piece_v(5) + mk_k(6) + piece_q(3) + piece_v(6)
            + mk_k(7) + piece_v(7))

    # Startup: k0/q0 feed four 512-wide j0 score groups (their exps start
    # while x1 is still in flight), then q1 unlocks the j1 halves.
    for f in mk_k(0) + piece_q(0):
        f()
    for i in range(4):
        emit_score512(0, i, 0)
    for f in piece_q(1):
        f()
    for i in range(4):
        emit_score512(0, i, 1)

    emitted = 0
    for i in range(4, NKC):
        need_k = i // 4 + 1
        while emitted < len(rest) and k_done["n"] < need_k:
            rest[emitted]()
            emitted += 1
        emit_attn_score(0, i)
        if PV0_AT <= i < NH0 + PV0_AT:
            emit_attn_pv(0, i - PV0_AT)
        while emitted < len(rest) and emitted * 26 < len(rest) * (i - 3):
            rest[emitted]()
            emitted += 1
    while emitted < len(rest):
        rest[emitted]()
        emitted += 1
    flush_vt(0)
    proj_cm.__exit__(None, None, None)
    po1_holder["pool"] = po1_cm.__enter__()

    # Half 1: scores with own PVs LAG-gated plus one deferred-h0 PV per
    # step (reservoir drains by step ~23); h0's fold/epilogue runs mid-
    # stream once po0 closes, so only h1's epilogue remains in the tail.
    pv0 = list(range(NH0, NKC))
    for s in range(NKC):
        emit_attn_score(1, s, split_exp=(s == NKC - 1))
        if s == NKC - NCHAIN:
            # open h1's denominator-fold group in po0's (now free) banks:
            # chain partials fold right away, sT(1,29..31) as they exp
            prsR[1] = po_pool.tile([128, QH], F32, tag="po0", name="prsR1")
            for j in range(2):
                emit_fold_mms(1, j, [sTs[1, s]] + fold_terms(1),
                              start=True, stop=False)
        elif s > NKC - NCHAIN and s < NKC - 1:
            for j in range(2):
                emit_fold_mms(1, j, [sTs[1, s]], start=False, stop=False)
        if s >= LAG:
            emit_attn_pv(1, s - LAG)
        for _ in range(POPS):
            if pv0:
                emit_attn_pv(0, pv0.pop(0))
        if s == FOLD0_AT:
            prsR[0] = pss_pool.tile([128, QH], F32, tag="pss", name="prsR0")
            for j in range(2):
                emit_fold_mms(0, j, fold_terms(0)
                              + [sTs[0, i] for i in range(NKC - NCHAIN, NKC)],
                              start=True, stop=True)
            emit_epi_finish(0)
    for s in range(NKC - LAG, NKC):
        emit_attn_pv(1, s)
    emit_epi_finish(1)


def build_bass(iters=1):
    nc = bacc.Bacc()
    xt = nc.dram_tensor("xt_part", [128, NCH * XCH], F8,
                        kind="ExternalInput")
    wcat = nc.dram_tensor("wcat", [128, 2 * 3 * ND * N], F8,
                          kind="ExternalInput")
    out = nc.dram_tensor("out_part", [128, (SQ // 128) * L], F32,
                         kind="ExternalOutput")
    with tile.TileContext(nc) as tc:
        for _ in range(iters):
            with ExitStack() as ctx:
                emit(nc, tc, ctx, xt, wcat, out)
    nc.compile()
    return nc


def make_in_maps(x, Wq, Wk, Wv):
    # wcat[p, (m two d n)] = W32m.T[d*128+p, n], two = (hi, lo), W32 = 32*W
    blocks = []
    for W in (Wq, Wk, Wv):
        w32 = np.asarray(W, np.float32).T * 32.0          # [D, n]
        wh = w32.astype(E4)
        wl = (w32 - wh.astype(np.float32)).astype(E4)
        for w8 in (wh, wl):
            wt = w8.reshape(ND, 128, N)                   # [d, p, n]
            blocks.append(wt.transpose(1, 0, 2).reshape(128, ND * N))
    wcat = np.ascontiguousarray(
        np.concatenate(blocks, axis=1))                   # [128, 2*3*ND*N]
    x = np.asarray(x, np.float32)
    in_maps = []
    for c in range(NCORES):
        bb, h = c // 2, c % 2
        xb = x[bb]
        x_part = xb if h == 0 else np.concatenate([xb[SQ:], xb[:SQ]], axis=0)
        # xt[p, (c d two s)] = fp8 hi/lo of x_part[c*512+s, d*128+p]
        xr = x_part.reshape(NCH, CCH, ND, 128).transpose(3, 0, 2, 1)
        xh = xr.astype(E4)                                # [p, c, d, s]
        xl = (xr - xh.astype(np.float32)).astype(E4)
        xt_part = np.ascontiguousarray(
            np.stack([xh, xl], axis=3)                    # [p, c, d, two, s]
            .reshape(128, NCH * XCH))
        in_maps.append({"xt_part": xt_part, "wcat": wcat})
    return in_maps


def kernel(x, Wq, Wk, Wv):
    nc = build_bass()
    res = run_bass_kernel_spmd(nc, make_in_maps(x, Wq, Wk, Wv),
                               core_ids=list(range(NCORES)))
    out = np.empty((B, S, L), dtype=np.float32)
    for c in range(NCORES):
        bb, h = c // 2, c % 2
        # device layout out_dev[l, q]: final rows are columns
        out[bb, h * SQ:(h + 1) * SQ] = res.results[c]["out_part"].T
    return out


# revision 69
# speedup vs baseline: 1.2805x; 1.0369x over previous
"""Single-head attention (B=4, S=4096, D=1024, N=L=128) on 8 trn2 NeuronCores.

Sharding: core c handles batch b = c//2, query half h = c%2 (2048 queries).
Each core receives the full context of its batch with its own query half
ordered FIRST (attention is permutation-invariant over the context axis).

fp8 strategy (per-stage, validated numerically against the f64 reference):
  - Projections run as fp8 DoubleRow matmuls (0.5 cycles/out-col, 2 planes
    of 128 contraction each) in THREE passes: x_hi@W32h (+x_lo plane fused),
    then x_hi@W32l over d-tile pairs. W32 = 32*W is pre-scaled on the host so
    its fp8 encoding avoids the e4m3 subnormal floor (sigma_W = 1/32); the
    32x is folded into the exp scale (q,k) / final epilogue multiply (v).
    Host supplies x as interleaved fp8 (hi, lo) residual pairs, so a proj
    chunk is 8 DR matmuls (pass12: planes = (x_hi_d, x_lo_d) vs duplicated
    W32h_d) + 4 DR matmuls (pass3: planes = d-tile pairs of x_hi vs W32l).
    12*256 cycles vs bf16's 8*512: 25% cheaper at bf16-level accuracy.
  - Scores run as fp8 DoubleRow with stationary [k_hi | k_hi] (stride-0
    plane broadcast) and moving [q_hi | q_lo]: full-precision q times fp8 k
    at 2x bf16 rate. Only the single k quantization (~2.4% rms) enters the
    softmax logits; measured end-to-end rel err ~9e-3 (gate 2e-2).
  - exp on ACT with scale = 1/(sqrt(D)*1024) (q,k both carry 32x).
  - PV stays bf16 (fp8 on either side measured 2-3e-2: over the gate).
Per-engine busy (cost model): PE ~69us, ACT (exp) ~66us, DVE ~55us.

Per-core pipeline (single interleaved emission). The schedule balances PE
against the ACT exp stream (64 x [128,1024] exps ~= 66us, the hard floor):
half 0 runs proj + scores only (PE ~34us ~= ACT), deferring ALL of half 0's
PV matmuls; half 1 runs scores + two PV groups per step (h0's deferred PVs
drain first, then h1's own) so PE stays fed at ~ACT rate throughout. All
half-0 sT tiles stay live in SBUF until their deferred PVs consume them.
PSUM: proj(2 banks)+pss(4) in half 0; pss(4)+po0(2)+po1(2) in half 1 (proj
pool released first); the denominator fold lands in pss-tagged tiles at the
tail. Softmax denominator partials accumulate as bf16 adds on DVE (chains
0-2) / gpsimd (chain 3, SBUF only - gpsimd cannot touch PSUM); chains
pre-fold on DVE then one ones-matmul broadcasts the denominator row across
partitions; epilogue does DVE reciprocal then a single scalar_tensor_tensor
(po * 1/32) * recip before the output DMA.
"""
from contextlib import ExitStack

import numpy as np
import ml_dtypes

import concourse.tile as tile
import concourse.mybir as mybir
from concourse import bacc
from concourse.bass_utils import run_bass_kernel_spmd

B, S, D, N, L = 4, 4096, 1024, 128, 128
NCORES = 8
SQ = B * S // NCORES      # 2048 queries per core
CCH = 512                 # projection chunk (tokens)
NCH = S // CCH            # 8 projection chunks
NKC = S // 128            # 32 kctx subchunks of 128
QH = 1024                 # query half processed per attention sweep
ND = D // 128             # 8 contraction tiles over D
NCHAIN = 4                # denominator partial chains
SCALE = 1.0 / float(np.sqrt(D))
EXP_SCALE = SCALE / 1024.0   # q,k each carry a 32x from W32 host pre-scale

BF = mybir.dt.bfloat16
F32 = mybir.dt.float32
F8 = mybir.dt.float8e4
E4 = ml_dtypes.float8_e4m3
NWARM = 24
UPFRONT_X = 4
Q_ACT_COPY = True
PV0_AT = 14
POPS = 1
FOLD0_AT = 26

DR = mybir.MatmulPerfMode.DoubleRow

# xt8 element offsets: [p, (c d two s)], two = (hi, lo)
XCH = ND * 2 * CCH        # elems per chunk per partition (8192)


def emit(nc, tc, ctx, xt, wcat, out):
    UPFRONT_X_ = UPFRONT_X
    persist = ctx.enter_context(tc.tile_pool(name="persist", bufs=1))
    zwarm = persist.tile([128, 128], BF, tag="zwarm")
    nc.gpsimd.memset(zwarm, 0.0)
    ones_mat = persist.tile([128, 128], BF, tag="ones_mat")
    nc.vector.memset(ones_mat, 1.0)

    xbig = persist.tile([128, NCH * XCH], F8, tag="xbig")
    # wcat: W32h [p, (m d n)] then W32l [p, (m d n)], m = (q, k, v)
    WSZ = 3 * ND * N
    wsb = persist.tile([128, 2 * WSZ], F8, tag="wsb")

    # HBM reads serialize at ~343 GB/s, so ALL loads ride one queue (SP) in
    # exactly the order the pipeline consumes them: Wq, x0, Wk, x1 (feeding
    # q0/k0/q1 and the first score group), then Wv and the remaining chunks.
    def xdma(c, parts=1):
        w = XCH // parts
        for s in range(parts):
            sl = slice(c * XCH + s * w, c * XCH + (s + 1) * w)
            nc.sync.dma_start(out=xbig[:, sl], in_=xt[:, sl])

    W1 = 2 * ND * N   # per-matrix (hi | lo) block
    half = XCH // 2

    def wdma(m):
        nc.sync.dma_start(out=wsb[:, m * W1:(m + 1) * W1],
                          in_=wcat[:, m * W1:(m + 1) * W1])

    # x6/x7 are issued lazily from the piece weave so the early vv XBAR
    # transposes (same SP queue) interleave with them instead of queuing
    # behind the whole x stream.
    wdma(1)
    xdma(0, parts=4)
    wdma(0)
    xdma(1, parts=2)
    wdma(2)
    for c in range(2, 2 + UPFRONT_X):
        xdma(c)

    def wh2(m, d):
        # stationary [128, 2, 128]: duplicated W32h_d planes (stride-0)
        off = m * W1 + d * N
        return wsb[:, off:off + N].unsqueeze(1).broadcast_to((128, 2, N))

    def wl2(m, t):
        # stationary [128, 2, 128]: planes (W32l_{2t}, W32l_{2t+1})
        off = m * W1 + ND * N + 2 * t * N
        return wsb[:, off:off + 2 * N].rearrange("p (two n) -> p two n",
                                                 two=2)

    def x12(c, d):
        # moving [128, 2, 512]: planes (x_hi_d, x_lo_d), contiguous
        off = c * XCH + d * 2 * CCH
        return xbig[:, off:off + 2 * CCH].rearrange(
            "p (two s) -> p two s", two=2)

    def x3(c, t):
        # moving [128, 2, 512]: planes (x_hi_{2t}, x_hi_{2t+1}), d-stride 1024
        off = c * XCH + 2 * t * 2 * CCH
        return xbig[:, off:off + 3 * CCH].rearrange(
            "p (d s) -> p d s", d=3)[:, 0::2, :]

    def proj_mm_a(ps, m, c):
        for d in range(ND):
            nc.tensor.matmul(ps, wh2(m, d), x12(c, d),
                             start=(d == 0), stop=False, perf_mode=DR)

    def proj_mm_b(ps, m, c):
        for t in range(ND // 2):
            nc.tensor.matmul(ps, wl2(m, t), x3(c, t),
                             start=False, stop=(t == ND // 2 - 1),
                             perf_mode=DR)

    kT8 = persist.tile([128, S], F8, tag="kT8")     # [n, kctx] fp8 (32x)
    vv = persist.tile([128, S], BF, tag="vv")       # 32 chunks [kctx128, l]
    qhl = persist.tile([128, 2 * SQ], F8, tag="qhl")  # per 512-q: (hi, lo)

    vtc_pool = ctx.enter_context(tc.tile_pool(name="vtc", bufs=3))

    # ---------------- attention pools (outer; proj pool nests inside) ----
    spool = ctx.enter_context(tc.tile_pool(name="sT", bufs=1))
    parts = ctx.enter_context(tc.tile_pool(name="parts", bufs=1))
    pss_pool = ctx.enter_context(tc.tile_pool(name="pss", bufs=2, space="PSUM"))
    po_pool = ctx.enter_context(tc.tile_pool(name="po", bufs=1, space="PSUM"))
    epi_sb = ctx.enter_context(tc.tile_pool(name="episb", bufs=2))

    proj_cm = tc.tile_pool(name="proj", bufs=2, space="PSUM")
    proj_ps = proj_cm.__enter__()
    po1_cm = tc.tile_pool(name="po1", bufs=1, space="PSUM")
    po1_holder = {"pool": None}

    # PE warmup while the first DMAs land (p-state ramp off critical path);
    # sized to end as x0 arrives so proj starts immediately at full rate
    for _ in range(NWARM):
        pwarm = proj_ps.tile([128, CCH], F32, tag="proj", name="pwarm")
        nc.tensor.matmul(pwarm[:, 0:128], zwarm, zwarm, start=True,
                         stop=True)

    vt_pending = []

    def flush_vt(n=None):
        while vt_pending and (n is None or len(vt_pending) > n):
            csl, vTc = vt_pending.pop(0)
            nc.sync.dma_start_transpose(
                out=vv[:, csl].rearrange("p (t q) -> p t q", t=CCH // 128),
                in_=vTc)

    # Each proj piece is emitted as two parts (pass12 / pass3+copies) so a
    # pending score group never waits behind a full 12-matmul chain.
    def piece_k(c):
        cell = {}
        csl = slice(c * CCH, (c + 1) * CCH)

        def a():
            if 2 + UPFRONT_X <= c + 2 < NCH:
                xdma(c + 2)
            cell["ps"] = proj_ps.tile([128, CCH], F32, tag="proj", name="pk")
            proj_mm_a(cell["ps"], 1, c)

        def b():
            proj_mm_b(cell["ps"], 1, c)
            # split the quantize so the first kctx subchunks unlock early
            h = CCH // 2
            nc.vector.tensor_copy(kT8[:, csl][:, 0:h], cell["ps"][:, 0:h])
            nc.vector.tensor_copy(kT8[:, csl][:, h:], cell["ps"][:, h:])
        return [a, b]

    def piece_v(c):
        cell = {}
        csl = slice(c * CCH, (c + 1) * CCH)

        def a():
            cell["ps"] = proj_ps.tile([128, CCH], F32, tag="proj", name="pv")
            proj_mm_a(cell["ps"], 2, c)

        def b():
            proj_mm_b(cell["ps"], 2, c)
            vTc = vtc_pool.tile([128, CCH], BF, tag="vTc", name=f"vTc{c % 3}")
            nc.vector.tensor_copy(vTc, cell["ps"])
            # defer the XBAR transpose issue one piece so it queues behind
            # the next x chunk's transfer on the SP DMA queue
            vt_pending.append((csl, vTc))
            flush_vt(1)
        return [a, b]

    def piece_q(c):
        cell = {}
        hi = slice(c * 2 * CCH, c * 2 * CCH + CCH)
        lo = slice(c * 2 * CCH + CCH, (c + 1) * 2 * CCH)

        def a():
            cell["ps"] = proj_ps.tile([128, CCH], F32, tag="proj", name="pq")
            proj_mm_a(cell["ps"], 0, c)

        def b():
            proj_mm_b(cell["ps"], 0, c)
            if c < 2 and Q_ACT_COPY:
                nc.scalar.activation(qhl[:, hi], cell["ps"],
                                     func=mybir.ActivationFunctionType.Copy)
            else:
                nc.vector.tensor_copy(qhl[:, hi], cell["ps"])
            nc.vector.tensor_tensor(out=qhl[:, lo], in0=cell["ps"],
                                    in1=qhl[:, hi],
                                    op=mybir.AluOpType.subtract)
        return [a, b]

    po = [None, None]
    part = {}
    sTs = {}
    NH0 = 8   # PV groups executed within half 0 (rest defer into half 1)
    LAG = 3   # PV trails its score by LAG steps (exp-gated pacing)

    def score_mm(h, i, j, out):
        ksl = slice(i * 128, (i + 1) * 128)
        kst = kT8[:, ksl].unsqueeze(1).broadcast_to((128, 2, 128))
        blk = h * 2 + j
        qmv = qhl[:, blk * 2 * CCH:(blk + 1) * 2 * CCH].rearrange(
            "p (two s) -> p two s", two=2)
        nc.tensor.matmul(out, kst, qmv, start=True, stop=True,
                         perf_mode=DR, skip_group_check=True)

    def sT_tag(h, i):
        # every h0 sT gets a dedicated buffer (PVs may run much later than
        # the score);  h1 rotates 8 bufs (PV trails by <= 8 there)
        return f"sTd{i}" if h == 0 else f"sTb{i % 8}"

    s512_ps = {}

    def emit_score512(h, i, j):
        # startup path: one j-block at a time (j0 needs only k0/q0); pss
        # tiles are shared by i-pairs so the pool rotation stays 2-deep
        key = (i // 2, j)
        if key not in s512_ps:
            s512_ps[key] = pss_pool.tile([128, QH], F32, tag="pss",
                                         name=f"pss512_{key}")
        ps = s512_ps[key][:, (i % 2) * CCH:(i % 2 + 1) * CCH]
        score_mm(h, i, j, ps)
        if j == 0:
            tg = sT_tag(h, i)
            sTs[h, i] = spool.tile([128, QH], BF, tag=tg, name=tg)
        sT = sTs[h, i]
        jsl = slice(j * (QH // 2), (j + 1) * (QH // 2))
        nc.scalar.activation(sT[:, jsl], ps,
                             func=mybir.ActivationFunctionType.Exp,
                             scale=EXP_SCALE)
        sTs[h, i] = sT
        if j == 1:
            emit_chain(h, i, sT)

    def emit_attn_score(h, i, split_exp=False):
        pss = pss_pool.tile([128, QH], F32, tag="pss")
        for j in range(2):
            score_mm(h, i, j,
                     pss[:, j * (QH // 2):(j + 1) * (QH // 2)])
        tg = sT_tag(h, i)
        sT = spool.tile([128, QH], BF, tag=tg, name=tg)
        sTs[h, i] = sT
        if split_exp:
            # last group of a half: per-j exps, each immediately folding the
            # denominator (3 ones-matmuls: c0, c2, this sT) so recip(j0)
            # starts one half-exp early
            prsR[h] = pss_pool.tile([128, QH], F32, tag="pss",
                                    name=f"prsR{h}")
            for j in range(2):
                jsl = slice(j * (QH // 2), (j + 1) * (QH // 2))
                nc.scalar.activation(sT[:, jsl], pss[:, jsl],
                                     func=mybir.ActivationFunctionType.Exp,
                                     scale=EXP_SCALE)
                emit_fold_mms(h, j, [part[h, 0], part[h, 2], sT],
                              start=True, stop=True)
            return
        nc.scalar.activation(sT, pss, func=mybir.ActivationFunctionType.Exp,
                             scale=EXP_SCALE)
        emit_chain(h, i, sT)

    def emit_chain(h, i, sT):
        # chains 2,3 ride gpsimd (SBUF-only engine, otherwise idle) except
        # near the end of a half so the fold is never gated on its slow
        # queue. The last 4 steps skip the vector add entirely: their sT
        # tiles feed the denominator fold matmuls directly.
        if i == NKC - 1:
            return  # sT(31) feeds the denominator fold directly
        ch = i % NCHAIN
        eng = nc.gpsimd if (ch >= 2 and i < NKC - 8) else nc.vector
        if i < NCHAIN:
            part[h, ch] = parts.tile([128, QH], BF, tag=f"part{h}_{ch}",
                                     name=f"part{h}_{ch}")
            eng.tensor_copy(part[h, ch], sT)
        elif i == NKC - 2:
            # j-split so c2's j0 half closes one half-add earlier
            for j in range(2):
                jsl = slice(j * (QH // 2), (j + 1) * (QH // 2))
                eng.tensor_add(part[h, ch][:, jsl], part[h, ch][:, jsl],
                               sT[:, jsl])
        else:
            eng.tensor_add(part[h, ch], part[h, ch], sT)

    def emit_attn_pv(h, i):
        ksl = slice(i * 128, (i + 1) * 128)
        if i == 0:
            # po0 shares half-0's psum budget with proj; po1's pool is
            # entered only after the proj pool releases its banks
            pool = po_pool if h == 0 else po1_holder["pool"]
            po[h] = pool.tile([128, QH], F32, tag=f"po{h}", name=f"po{h}")
        for j in range(2):
            jsl = slice(j * (QH // 2), (j + 1) * (QH // 2))
            nc.tensor.matmul(po[h][:, jsl], vv[:, ksl], sTs[h, i][:, jsl],
                             start=(i == 0), stop=(i == NKC - 1),
                             skip_group_check=True)

    prsR = [None, None]

    # The denominator fold is pure PE work: ones-matmuls accumulate the four
    # chain partials plus the last four sT tiles directly into prsR (every
    # partition gets the full denominator row), so no vector adds sit between
    # the final exp and the reciprocal.
    def fold_terms(h):
        return [part[h, ch] for ch in range(NCHAIN)]

    def emit_fold_mms(h, j, terms, start, stop):
        jsl = slice(j * (QH // 2), (j + 1) * (QH // 2))
        for t, term in enumerate(terms):
            nc.tensor.matmul(prsR[h][:, jsl], ones_mat, term[:, jsl],
                             start=(start and t == 0),
                             stop=(stop and t == len(terms) - 1),
                             skip_group_check=True)

    def emit_epi_finish(h):
        for j in range(2):
            jsl = slice(j * (QH // 2), (j + 1) * (QH // 2))
            recipB = epi_sb.tile([128, QH // 2], F32, tag="recipB")
            nc.vector.reciprocal(recipB, prsR[h][:, jsl])
            foall = epi_sb.tile([128, QH // 2], F32, tag="foall",
                                name=f"foall{j}")
            # v carries a 32x from W32v: out = (po * 1/32) * recip; quarter
            # columns pipeline the multiply with the output DMA
            for q in range(2):
                qsl = slice(q * (QH // 4), (q + 1) * (QH // 4))
                base = h * QH + j * (QH // 2) + q * (QH // 4)
                nc.vector.scalar_tensor_tensor(
                    out=foall[:, qsl], in0=po[h][:, jsl][:, qsl],
                    scalar=1.0 / 32.0, in1=recipB[:, qsl],
                    op0=mybir.AluOpType.mult, op1=mybir.AluOpType.mult)
                nc.sync.dma_start(out=out[:, base:base + QH // 4],
                                  in_=foall[:, qsl])

    # ---------------- schedule ----------------
    # Half 0: proj piece-parts woven into the score steps (q0,k0,q1 up front
    # so the first exp lands early; k(c) hard-gated before step 4c; q2/q3
    # late). Only PV groups 0..NH0-1 run here (LAG-gated) — h0 stays just
    # under the ACT exp rate. The remaining PV groups defer into half 1 as a
    # ready reservoir the scheduler can pull into any PE idle slot.
    k_done = {"n": 0}

    def mk_k(c):
        a, b = piece_k(c)

        def b2():
            b()
            k_done["n"] = c + 1
        return [a, b2]

    rest = (mk_k(1) + piece_v(0) + piece_v(1) + mk_k(2) + piece_v(2)
            + mk_k(3) + piece_v(3) + mk_k(4) + piece_v(4) + mk_k(5)
            + piece_q(2) + piece_v(5) + mk_k(6) + piece_q(3) + piece_v(6)
            + mk_k(7) + piece_v(7))

    # Startup: k0/q0 feed four 512-wide j0 score groups (their exps start
    # while x1 is still in flight), then q1 unlocks the j1 halves.
    for f in mk_k(0) + piece_q(0):
        f()
    for i in range(4):
        emit_score512(0, i, 0)
    for f in piece_q(1):
        f()
    for i in range(4):
        emit_score512(0, i, 1)

    emitted = 0
    for i in range(4, NKC):
        need_k = i // 4 + 1
        while emitted < len(rest) and k_done["n"] < need_k:
            rest[emitted]()
            emitted += 1
        emit_attn_score(0, i)
        if PV0_AT <= i < NH0 + PV0_AT:
            emit_attn_pv(0, i - PV0_AT)
        while emitted < len(rest) and emitted * 26 < len(rest) * (i - 3):
            rest[emitted]()
            emitted += 1
    while emitted < len(rest):
        rest[emitted]()
        emitted += 1
    flush_vt(0)
    proj_cm.__exit__(None, None, None)
    po1_holder["pool"] = po1_cm.__enter__()

    # Half 1: scores with own PVs LAG-gated plus one deferred-h0 PV per
    # step (reservoir drains by step ~23); h0's fold/epilogue runs mid-
    # stream once po0 closes, so only h1's epilogue remains in the tail.
    # h0's chains collapse on DVE (mid-stream, off the critical path)
    for ch in (1, 2, 3):
        nc.vector.tensor_add(part[0, 0], part[0, 0], part[0, ch])
    pv0 = list(range(NH0, NKC))
    for s in range(NKC):
        emit_attn_score(1, s, split_exp=(s == NKC - 1))
        if s == NKC - NCHAIN:
            # c3 closed at step NKC-5: collapse into c0 off the tail path
            nc.vector.tensor_add(part[1, 0], part[1, 0], part[1, 3])
        elif s == NKC - 3:
            nc.vector.tensor_add(part[1, 0], part[1, 0], part[1, 1])
        if s >= LAG:
            emit_attn_pv(1, s - LAG)
        for _ in range(POPS):
            if pv0:
                emit_attn_pv(0, pv0.pop(0))
        if s == FOLD0_AT:
            prsR[0] = pss_pool.tile([128, QH], F32, tag="pss", name="prsR0")
            for j in range(2):
                emit_fold_mms(0, j, [part[0, 0], sTs[0, NKC - 1]],
                              start=True, stop=True)
            emit_epi_finish(0)
    for s in range(NKC - LAG, NKC):
        emit_attn_pv(1, s)
    emit_epi_finish(1)


def build_bass(iters=1):
    nc = bacc.Bacc()
    xt = nc.dram_tensor("xt_part", [128, NCH * XCH], F8,
                        kind="ExternalInput")
    wcat = nc.dram_tensor("wcat", [128, 2 * 3 * ND * N], F8,
                          kind="ExternalInput")
    out = nc.dram_tensor("out_part", [128, (SQ // 128) * L], F32,
                         kind="ExternalOutput")
    with tile.TileContext(nc) as tc:
        for _ in range(iters):
            with ExitStack() as ctx:
                emit(nc, tc, ctx, xt, wcat, out)
    nc.compile()
    return nc


def make_in_maps(x, Wq, Wk, Wv):
    # wcat[p, (m two d n)] = W32m.T[d*128+p, n], two = (hi, lo), W32 = 32*W
    blocks = []
    for W in (Wq, Wk, Wv):
        w32 = np.asarray(W, np.float32).T * 32.0          # [D, n]
        wh = w32.astype(E4)
        wl = (w32 - wh.astype(np.float32)).astype(E4)
        for w8 in (wh, wl):
            wt = w8.reshape(ND, 128, N)                   # [d, p, n]
            blocks.append(wt.transpose(1, 0, 2).reshape(128, ND * N))
    wcat = np.ascontiguousarray(
        np.concatenate(blocks, axis=1))                   # [128, 2*3*ND*N]
    x = np.asarray(x, np.float32)
    in_maps = []
    for c in range(NCORES):
        bb, h = c // 2, c % 2
        xb = x[bb]
        x_part = xb if h == 0 else np.concatenate([xb[SQ:], xb[:SQ]], axis=0)
        # xt[p, (c d two s)] = fp8 hi/lo of x_part[c*512+s, d*128+p]
        xr = x_part.reshape(NCH, CCH, ND, 128).transpose(3, 0, 2, 1)
        xh = xr.astype(E4)                                # [p, c, d, s]
        xl = (xr - xh.astype(np.float32)).astype(E4)
        xt_part = np.ascontiguousarray(
            np.stack([xh, xl], axis=3)                    # [p, c, d, two, s]
            .reshape(128, NCH * XCH))
        in_maps.append({"xt_part": xt_part, "wcat": wcat})
    return in_maps


def kernel(x, Wq, Wk, Wv):
    nc = build_bass()
    res = run_bass_kernel_spmd(nc, make_in_maps(x, Wq, Wk, Wv),
                               core_ids=list(range(NCORES)))
    out = np.empty((B, S, L), dtype=np.float32)
    for c in range(NCORES):
        bb, h = c // 2, c % 2
        # device layout out_dev[l, q]: final rows are columns
        out[bb, h * SQ:(h + 1) * SQ] = res.results[c]["out_part"].T
    return out


# revision 82
# speedup vs baseline: 1.2912x; 1.0084x over previous
"""Single-head attention (B=4, S=4096, D=1024, N=L=128) on 8 trn2 NeuronCores.

Sharding: core c handles batch b = c//2, query half h = c%2 (2048 queries).
Each core receives the full context of its batch with its own query half
ordered FIRST (attention is permutation-invariant over the context axis).

fp8 strategy (per-stage, validated numerically against the f64 reference):
  - Projections run as fp8 DoubleRow matmuls (0.5 cycles/out-col, 2 planes
    of 128 contraction each) in THREE passes: x_hi@W32h (+x_lo plane fused),
    then x_hi@W32l over d-tile pairs. W32 = 32*W is pre-scaled on the host so
    its fp8 encoding avoids the e4m3 subnormal floor (sigma_W = 1/32); the
    32x is folded into the exp scale (q,k) / final epilogue multiply (v).
    Host supplies x as interleaved fp8 (hi, lo) residual pairs, so a proj
    chunk is 8 DR matmuls (pass12: planes = (x_hi_d, x_lo_d) vs duplicated
    W32h_d) + 4 DR matmuls (pass3: planes = d-tile pairs of x_hi vs W32l).
    12*256 cycles vs bf16's 8*512: 25% cheaper at bf16-level accuracy.
  - Scores run as fp8 DoubleRow with stationary [k_hi | k_hi] (stride-0
    plane broadcast) and moving [q_hi | q_lo]: full-precision q times fp8 k
    at 2x bf16 rate. Only the single k quantization (~2.4% rms) enters the
    softmax logits; measured end-to-end rel err ~9e-3 (gate 2e-2).
  - exp on ACT with scale = 1/(sqrt(D)*1024) (q,k both carry 32x).
  - PV stays bf16 (fp8 on either side measured 2-3e-2: over the gate).
Per-engine busy (cost model): PE ~69us, ACT (exp) ~66us, DVE ~55us.

Per-core pipeline (single interleaved emission). The schedule balances PE
against the ACT exp stream (64 x [128,1024] exps ~= 66us, the hard floor):
half 0 runs proj + scores only (PE ~34us ~= ACT), deferring ALL of half 0's
PV matmuls; half 1 runs scores + two PV groups per step (h0's deferred PVs
drain first, then h1's own) so PE stays fed at ~ACT rate throughout. All
half-0 sT tiles stay live in SBUF until their deferred PVs consume them.
PSUM: proj(2 banks)+pss(4) in half 0; pss(4)+po0(2)+po1(2) in half 1 (proj
pool released first); the denominator fold lands in pss-tagged tiles at the
tail. Softmax denominator partials accumulate as bf16 adds on DVE (chains
0-2) / gpsimd (chain 3, SBUF only - gpsimd cannot touch PSUM); chains
pre-fold on DVE then one ones-matmul broadcasts the denominator row across
partitions; epilogue does DVE reciprocal then a single scalar_tensor_tensor
(po * 1/32) * recip before the output DMA.
"""
from contextlib import ExitStack

import numpy as np
import ml_dtypes

import concourse.tile as tile
import concourse.mybir as mybir
from concourse import bacc
from concourse.bass_utils import run_bass_kernel_spmd

B, S, D, N, L = 4, 4096, 1024, 128, 128
NCORES = 8
SQ = B * S // NCORES      # 2048 queries per core
CCH = 512                 # projection chunk (tokens)
NCH = S // CCH            # 8 projection chunks
NKC = S // 128            # 32 kctx subchunks of 128
QH = 1024                 # query half processed per attention sweep
ND = D // 128             # 8 contraction tiles over D
NCHAIN = 4                # denominator partial chains
LAG_H1 = 3
SCALE = 1.0 / float(np.sqrt(D))
EXP_SCALE = SCALE / 1024.0   # q,k each carry a 32x from W32 host pre-scale

BF = mybir.dt.bfloat16
F32 = mybir.dt.float32
F8 = mybir.dt.float8e4
E4 = ml_dtypes.float8_e4m3
NWARM = 24
UPFRONT_X = 4
Q_ACT_COPY = True
PV0_AT = 14
POPS = 1
FOLD0_AT = 23
PACE = 26

DR = mybir.MatmulPerfMode.DoubleRow

# xt8 element offsets: [p, (c d two s)], two = (hi, lo)
XCH = ND * 2 * CCH        # elems per chunk per partition (8192)


def emit(nc, tc, ctx, xt, wcat, out):
    UPFRONT_X_ = UPFRONT_X
    persist = ctx.enter_context(tc.tile_pool(name="persist", bufs=1))
    zwarm = persist.tile([128, 128], BF, tag="zwarm")
    nc.gpsimd.memset(zwarm, 0.0)
    ones_mat = persist.tile([128, 128], BF, tag="ones_mat")
    nc.vector.memset(ones_mat, 1.0)

    xbig = persist.tile([128, NCH * XCH], F8, tag="xbig")
    # wcat: W32h [p, (m d n)] then W32l [p, (m d n)], m = (q, k, v)
    WSZ = 3 * ND * N
    wsb = persist.tile([128, 2 * WSZ], F8, tag="wsb")

    # HBM reads serialize at ~343 GB/s, so ALL loads ride one queue (SP) in
    # exactly the order the pipeline consumes them: Wq, x0, Wk, x1 (feeding
    # q0/k0/q1 and the first score group), then Wv and the remaining chunks.
    def xdma(c, parts=1):
        w = XCH // parts
        for s in range(parts):
            sl = slice(c * XCH + s * w, c * XCH + (s + 1) * w)
            nc.sync.dma_start(out=xbig[:, sl], in_=xt[:, sl])

    W1 = 2 * ND * N   # per-matrix (hi | lo) block
    half = XCH // 2

    def wdma(m):
        nc.sync.dma_start(out=wsb[:, m * W1:(m + 1) * W1],
                          in_=wcat[:, m * W1:(m + 1) * W1])

    # x6/x7 are issued lazily from the piece weave so the early vv XBAR
    # transposes (same SP queue) interleave with them instead of queuing
    # behind the whole x stream.
    wdma(1)
    xdma(0, parts=4)
    wdma(0)
    xdma(1, parts=2)
    wdma(2)
    for c in range(2, 2 + UPFRONT_X):
        xdma(c)

    def wh2(m, d):
        # stationary [128, 2, 128]: duplicated W32h_d planes (stride-0)
        off = m * W1 + d * N
        return wsb[:, off:off + N].unsqueeze(1).broadcast_to((128, 2, N))

    def wl2(m, t):
        # stationary [128, 2, 128]: planes (W32l_{2t}, W32l_{2t+1})
        off = m * W1 + ND * N + 2 * t * N
        return wsb[:, off:off + 2 * N].rearrange("p (two n) -> p two n",
                                                 two=2)

    def x12(c, d):
        # moving [128, 2, 512]: planes (x_hi_d, x_lo_d), contiguous
        off = c * XCH + d * 2 * CCH
        return xbig[:, off:off + 2 * CCH].rearrange(
            "p (two s) -> p two s", two=2)

    def x3(c, t):
        # moving [128, 2, 512]: planes (x_hi_{2t}, x_hi_{2t+1}), d-stride 1024
        off = c * XCH + 2 * t * 2 * CCH
        return xbig[:, off:off + 3 * CCH].rearrange(
            "p (d s) -> p d s", d=3)[:, 0::2, :]

    def proj_mm_a(ps, m, c):
        for d in range(ND):
            nc.tensor.matmul(ps, wh2(m, d), x12(c, d),
                             start=(d == 0), stop=False, perf_mode=DR)

    def proj_mm_b(ps, m, c):
        for t in range(ND // 2):
            nc.tensor.matmul(ps, wl2(m, t), x3(c, t),
                             start=False, stop=(t == ND // 2 - 1),
                             perf_mode=DR)

    kT8 = persist.tile([128, S], F8, tag="kT8")     # [n, kctx] fp8 (32x)
    vv = persist.tile([128, S], BF, tag="vv")       # 32 chunks [kctx128, l]
    qhl = persist.tile([128, 2 * SQ], F8, tag="qhl")  # per 512-q: (hi, lo)

    vtc_pool = ctx.enter_context(tc.tile_pool(name="vtc", bufs=3))

    # ---------------- attention pools (outer; proj pool nests inside) ----
    spool = ctx.enter_context(tc.tile_pool(name="sT", bufs=1))
    parts = ctx.enter_context(tc.tile_pool(name="parts", bufs=1))
    pss_pool = ctx.enter_context(tc.tile_pool(name="pss", bufs=2, space="PSUM"))
    po_pool = ctx.enter_context(tc.tile_pool(name="po", bufs=1, space="PSUM"))
    epi_sb = ctx.enter_context(tc.tile_pool(name="episb", bufs=2))

    proj_cm = tc.tile_pool(name="proj", bufs=2, space="PSUM")
    proj_ps = proj_cm.__enter__()
    po1_cm = tc.tile_pool(name="po1", bufs=1, space="PSUM")
    po1_holder = {"pool": None}

    # PE warmup while the first DMAs land (p-state ramp off critical path);
    # sized to end as x0 arrives so proj starts immediately at full rate
    for _ in range(NWARM):
        pwarm = proj_ps.tile([128, CCH], F32, tag="proj", name="pwarm")
        nc.tensor.matmul(pwarm[:, 0:128], zwarm, zwarm, start=True,
                         stop=True)

    vt_pending = []

    def flush_vt(n=None):
        while vt_pending and (n is None or len(vt_pending) > n):
            csl, vTc = vt_pending.pop(0)
            nc.sync.dma_start_transpose(
                out=vv[:, csl].rearrange("p (t q) -> p t q", t=CCH // 128),
                in_=vTc)

    # Each proj piece is emitted as two parts (pass12 / pass3+copies) so a
    # pending score group never waits behind a full 12-matmul chain.
    def piece_k(c):
        cell = {}
        csl = slice(c * CCH, (c + 1) * CCH)

        def a():
            if 2 + UPFRONT_X <= c + 2 < NCH:
                xdma(c + 2)
            cell["ps"] = proj_ps.tile([128, CCH], F32, tag="proj", name="pk")
            proj_mm_a(cell["ps"], 1, c)

        def b():
            proj_mm_b(cell["ps"], 1, c)
            # split the quantize so the first kctx subchunks unlock early
            h = CCH // 2
            nc.vector.tensor_copy(kT8[:, csl][:, 0:h], cell["ps"][:, 0:h])
            nc.vector.tensor_copy(kT8[:, csl][:, h:], cell["ps"][:, h:])
        return [a, b]

    def piece_v(c):
        cell = {}
        csl = slice(c * CCH, (c + 1) * CCH)

        def a():
            cell["ps"] = proj_ps.tile([128, CCH], F32, tag="proj", name="pv")
            proj_mm_a(cell["ps"], 2, c)

        def b():
            proj_mm_b(cell["ps"], 2, c)
            vTc = vtc_pool.tile([128, CCH], BF, tag="vTc", name=f"vTc{c % 3}")
            nc.vector.tensor_copy(vTc, cell["ps"])
            # defer the XBAR transpose issue one piece so it queues behind
            # the next x chunk's transfer on the SP DMA queue
            vt_pending.append((csl, vTc))
            flush_vt(1)
        return [a, b]

    def piece_q(c):
        cell = {}
        hi = slice(c * 2 * CCH, c * 2 * CCH + CCH)
        lo = slice(c * 2 * CCH + CCH, (c + 1) * 2 * CCH)

        def a():
            cell["ps"] = proj_ps.tile([128, CCH], F32, tag="proj", name="pq")
            proj_mm_a(cell["ps"], 0, c)

        def b():
            proj_mm_b(cell["ps"], 0, c)
            if c < 2 and Q_ACT_COPY:
                nc.scalar.activation(qhl[:, hi], cell["ps"],
                                     func=mybir.ActivationFunctionType.Copy)
            else:
                nc.vector.tensor_copy(qhl[:, hi], cell["ps"])
            nc.vector.tensor_tensor(out=qhl[:, lo], in0=cell["ps"],
                                    in1=qhl[:, hi],
                                    op=mybir.AluOpType.subtract)
        return [a, b]

    po = [None, None]
    part = {}
    sTs = {}
    NH0 = 8   # PV groups executed within half 0 (rest defer into half 1)
    LAG = LAG_H1   # PV trails its score by LAG steps (exp-gated pacing)

    def score_mm(h, i, j, out):
        ksl = slice(i * 128, (i + 1) * 128)
        kst = kT8[:, ksl].unsqueeze(1).broadcast_to((128, 2, 128))
        blk = h * 2 + j
        qmv = qhl[:, blk * 2 * CCH:(blk + 1) * 2 * CCH].rearrange(
            "p (two s) -> p two s", two=2)
        nc.tensor.matmul(out, kst, qmv, start=True, stop=True,
                         perf_mode=DR, skip_group_check=True)

    def sT_tag(h, i):
        # every h0 sT gets a dedicated buffer (PVs may run much later than
        # the score);  h1 rotates 8 bufs (PV trails by <= 8 there)
        return f"sTd{i}" if h == 0 else f"sTb{i % 8}"

    s512_ps = {}

    def emit_score512(h, i, j):
        # startup path: one j-block at a time (j0 needs only k0/q0); pss
        # tiles are shared by i-pairs so the pool rotation stays 2-deep
        key = (i // 2, j)
        if key not in s512_ps:
            s512_ps[key] = pss_pool.tile([128, QH], F32, tag="pss",
                                         name=f"pss512_{key}")
        ps = s512_ps[key][:, (i % 2) * CCH:(i % 2 + 1) * CCH]
        score_mm(h, i, j, ps)
        if j == 0:
            tg = sT_tag(h, i)
            sTs[h, i] = spool.tile([128, QH], BF, tag=tg, name=tg)
        sT = sTs[h, i]
        jsl = slice(j * (QH // 2), (j + 1) * (QH // 2))
        nc.scalar.activation(sT[:, jsl], ps,
                             func=mybir.ActivationFunctionType.Exp,
                             scale=EXP_SCALE)
        sTs[h, i] = sT
        if j == 1:
            emit_chain(h, i, sT)

    def emit_attn_score(h, i, split_exp=False):
        pss = pss_pool.tile([128, QH], F32, tag="pss")
        for j in range(2):
            score_mm(h, i, j,
                     pss[:, j * (QH // 2):(j + 1) * (QH // 2)])
        tg = sT_tag(h, i)
        sT = spool.tile([128, QH], BF, tag=tg, name=tg)
        sTs[h, i] = sT
        if split_exp:
            # last group of a half: per-j exps, each immediately folding the
            # denominator (3 ones-matmuls: c0, c2, this sT) so recip(j0)
            # starts one half-exp early
            prsR[h] = pss_pool.tile([128, QH], F32, tag="pss",
                                    name=f"prsR{h}")
            for j in range(2):
                jsl = slice(j * (QH // 2), (j + 1) * (QH // 2))
                nc.scalar.activation(sT[:, jsl], pss[:, jsl],
                                     func=mybir.ActivationFunctionType.Exp,
                                     scale=EXP_SCALE)
                emit_fold_mms(h, j, [part[h, 0], part[h, 2], sT],
                              start=True, stop=True)
            return
        nc.scalar.activation(sT, pss, func=mybir.ActivationFunctionType.Exp,
                             scale=EXP_SCALE)
        emit_chain(h, i, sT)

    def emit_chain(h, i, sT):
        # chains 2,3 ride gpsimd (SBUF-only engine, otherwise idle) except
        # near the end of a half so the fold is never gated on its slow
        # queue. The last 4 steps skip the vector add entirely: their sT
        # tiles feed the denominator fold matmuls directly.
        if i == NKC - 1:
            return  # sT(31) feeds the denominator fold directly
        ch = i % NCHAIN
        eng = nc.gpsimd if (ch >= 2 and i < NKC - 8) else nc.vector
        if i < NCHAIN:
            part[h, ch] = parts.tile([128, QH], BF, tag=f"part{h}_{ch}",
                                     name=f"part{h}_{ch}")
            eng.tensor_copy(part[h, ch], sT)
        elif i == NKC - 2:
            # j-split so c2's j0 half closes one half-add earlier
            for j in range(2):
                jsl = slice(j * (QH // 2), (j + 1) * (QH // 2))
                eng.tensor_add(part[h, ch][:, jsl], part[h, ch][:, jsl],
                               sT[:, jsl])
        else:
            eng.tensor_add(part[h, ch], part[h, ch], sT)

    def emit_attn_pv(h, i):
        ksl = slice(i * 128, (i + 1) * 128)
        if i == 0:
            # po0 shares half-0's psum budget with proj; po1's pool is
            # entered only after the proj pool releases its banks
            pool = po_pool if h == 0 else po1_holder["pool"]
            po[h] = pool.tile([128, QH], F32, tag=f"po{h}", name=f"po{h}")
        for j in range(2):
            jsl = slice(j * (QH // 2), (j + 1) * (QH // 2))
            nc.tensor.matmul(po[h][:, jsl], vv[:, ksl], sTs[h, i][:, jsl],
                             start=(i == 0), stop=(i == NKC - 1),
                             skip_group_check=True)

    prsR = [None, None]

    # The denominator fold is pure PE work: ones-matmuls accumulate the four
    # chain partials plus the last four sT tiles directly into prsR (every
    # partition gets the full denominator row), so no vector adds sit between
    # the final exp and the reciprocal.
    def fold_terms(h):
        return [part[h, ch] for ch in range(NCHAIN)]

    def emit_fold_mms(h, j, terms, start, stop):
        jsl = slice(j * (QH // 2), (j + 1) * (QH // 2))
        for t, term in enumerate(terms):
            nc.tensor.matmul(prsR[h][:, jsl], ones_mat, term[:, jsl],
                             start=(start and t == 0),
                             stop=(stop and t == len(terms) - 1),
                             skip_group_check=True)

    def emit_epi_finish(h):
        for j in range(2):
            jsl = slice(j * (QH // 2), (j + 1) * (QH // 2))
            recipB = epi_sb.tile([128, QH // 2], F32, tag="recipB")
            nc.vector.reciprocal(recipB, prsR[h][:, jsl])
            foall = epi_sb.tile([128, QH // 2], F32, tag="foall",
                                name=f"foall{j}")
            # v carries a 32x from W32v: out = (po * 1/32) * recip; quarter
            # columns pipeline the multiply with the output DMA
            for q in range(2):
                qsl = slice(q * (QH // 4), (q + 1) * (QH // 4))
                base = h * QH + j * (QH // 2) + q * (QH // 4)
                nc.vector.scalar_tensor_tensor(
                    out=foall[:, qsl], in0=po[h][:, jsl][:, qsl],
                    scalar=1.0 / 32.0, in1=recipB[:, qsl],
                    op0=mybir.AluOpType.mult, op1=mybir.AluOpType.mult)
                nc.sync.dma_start(out=out[:, base:base + QH // 4],
                                  in_=foall[:, qsl])

    # ---------------- schedule ----------------
    # Half 0: proj piece-parts woven into the score steps (q0,k0,q1 up front
    # so the first exp lands early; k(c) hard-gated before step 4c; q2/q3
    # late). Only PV groups 0..NH0-1 run here (LAG-gated) — h0 stays just
    # under the ACT exp rate. The remaining PV groups defer into half 1 as a
    # ready reservoir the scheduler can pull into any PE idle slot.
    k_done = {"n": 0}

    def mk_k(c):
        a, b = piece_k(c)

        def b2():
            b()
            k_done["n"] = c + 1
        return [a, b2]

    rest = (mk_k(1) + piece_v(0) + piece_v(1) + mk_k(2) + piece_v(2)
            + mk_k(3) + piece_v(3) + mk_k(4) + piece_v(4) + mk_k(5)
            + piece_q(2) + piece_v(5) + mk_k(6) + piece_q(3) + piece_v(6)
            + mk_k(7) + piece_v(7))

    # Startup: k0/q0 feed four 512-wide j0 score groups (their exps start
    # while x1 is still in flight), then q1 unlocks the j1 halves.
    for f in mk_k(0) + piece_q(0):
        f()
    for i in range(4):
        emit_score512(0, i, 0)
    for f in piece_q(1):
        f()
    for i in range(4):
        emit_score512(0, i, 1)

    emitted = 0
    for i in range(4, NKC):
        need_k = i // 4 + 1
        while emitted < len(rest) and k_done["n"] < need_k:
            rest[emitted]()
            emitted += 1
        emit_attn_score(0, i)
        if PV0_AT <= i < NH0 + PV0_AT:
            emit_attn_pv(0, i - PV0_AT)
        while emitted < len(rest) and emitted * PACE < len(rest) * (i - 3):
            rest[emitted]()
            emitted += 1
    while emitted < len(rest):
        rest[emitted]()
        emitted += 1
    flush_vt(0)
    proj_cm.__exit__(None, None, None)
    po1_holder["pool"] = po1_cm.__enter__()

    # Half 1: scores with own PVs LAG-gated plus one deferred-h0 PV per
    # step (reservoir drains by step ~23); h0's fold/epilogue runs mid-
    # stream once po0 closes, so only h1's epilogue remains in the tail.
    # h0's chains collapse on DVE (mid-stream, off the critical path)
    for ch in (1, 2, 3):
        nc.vector.tensor_add(part[0, 0], part[0, 0], part[0, ch])
    pv0 = list(range(NH0, NKC))
    for s in range(NKC):
        emit_attn_score(1, s, split_exp=(s == NKC - 1))
        if s == NKC - NCHAIN:
            # c3 closed at step NKC-5: collapse into c0 off the tail path
            nc.vector.tensor_add(part[1, 0], part[1, 0], part[1, 3])
        elif s == NKC - 3:
            nc.vector.tensor_add(part[1, 0], part[1, 0], part[1, 1])
        if s >= LAG:
            emit_attn_pv(1, s - LAG)
        for _ in range(POPS):
            if pv0:
                emit_attn_pv(0, pv0.pop(0))
        if s == FOLD0_AT:
            prsR[0] = pss_pool.tile([128, QH], F32, tag="pss", name="prsR0")
            for j in range(2):
                emit_fold_mms(0, j, [part[0, 0], sTs[0, NKC - 1]],
                              start=True, stop=True)
            emit_epi_finish(0)
    for s in range(NKC - LAG, NKC):
        emit_attn_pv(1, s)
    emit_epi_finish(1)


def build_bass(iters=1):
    nc = bacc.Bacc()
    xt = nc.dram_tensor("xt_part", [128, NCH * XCH], F8,
                        kind="ExternalInput")
    wcat = nc.dram_tensor("wcat", [128, 2 * 3 * ND * N], F8,
                          kind="ExternalInput")
    out = nc.dram_tensor("out_part", [128, (SQ // 128) * L], F32,
                         kind="ExternalOutput")
    with tile.TileContext(nc) as tc:
        for _ in range(iters):
            with ExitStack() as ctx:
                emit(nc, tc, ctx, xt, wcat, out)
    nc.compile()
    return nc


def make_in_maps(x, Wq, Wk, Wv):
    # wcat[p, (m two d n)] = W32m.T[d*128+p, n], two = (hi, lo), W32 = 32*W
    blocks = []
    for W in (Wq, Wk, Wv):
        w32 = np.asarray(W, np.float32).T * 32.0          # [D, n]
        wh = w32.astype(E4)
        wl = (w32 - wh.astype(np.float32)).astype(E4)
        for w8 in (wh, wl):
            wt = w8.reshape(ND, 128, N)                   # [d, p, n]
            blocks.append(wt.transpose(1, 0, 2).reshape(128, ND * N))
    wcat = np.ascontiguousarray(
        np.concatenate(blocks, axis=1))                   # [128, 2*3*ND*N]
    x = np.asarray(x, np.float32)
    in_maps = []
    for c in range(NCORES):
        bb, h = c // 2, c % 2
        xb = x[bb]
        x_part = xb if h == 0 else np.concatenate([xb[SQ:], xb[:SQ]], axis=0)
        # xt[p, (c d two s)] = fp8 hi/lo of x_part[c*512+s, d*128+p]
        xr = x_part.reshape(NCH, CCH, ND, 128).transpose(3, 0, 2, 1)
        xh = xr.astype(E4)                                # [p, c, d, s]
        xl = (xr - xh.astype(np.float32)).astype(E4)
        xt_part = np.ascontiguousarray(
            np.stack([xh, xl], axis=3)                    # [p, c, d, two, s]
            .reshape(128, NCH * XCH))
        in_maps.append({"xt_part": xt_part, "wcat": wcat})
    return in_maps


def kernel(x, Wq, Wk, Wv):
    nc = build_bass()
    res = run_bass_kernel_spmd(nc, make_in_maps(x, Wq, Wk, Wv),
                               core_ids=list(range(NCORES)))
    out = np.empty((B, S, L), dtype=np.float32)
    for c in range(NCORES):
        bb, h = c // 2, c % 2
        # device layout out_dev[l, q]: final rows are columns
        out[bb, h * SQ:(h + 1) * SQ] = res.results[c]["out_part"].T
    return out


# revision 87
# speedup vs baseline: 1.2926x; 1.0010x over previous
"""Single-head attention (B=4, S=4096, D=1024, N=L=128) on 8 trn2 NeuronCores.

Sharding: core c handles batch b = c//2, query half h = c%2 (2048 queries).
Each core receives the full context of its batch with its own query half
ordered FIRST (attention is permutation-invariant over the context axis).

fp8 strategy (per-stage, validated numerically against the f64 reference):
  - Projections run as fp8 DoubleRow matmuls (0.5 cycles/out-col, 2 planes
    of 128 contraction each) in THREE passes: x_hi@W32h (+x_lo plane fused),
    then x_hi@W32l over d-tile pairs. W32 = 32*W is pre-scaled on the host so
    its fp8 encoding avoids the e4m3 subnormal floor (sigma_W = 1/32); the
    32x is folded into the exp scale (q,k) / final epilogue multiply (v).
    Host supplies x as interleaved fp8 (hi, lo) residual pairs, so a proj
    chunk is 8 DR matmuls (pass12: planes = (x_hi_d, x_lo_d) vs duplicated
    W32h_d) + 4 DR matmuls (pass3: planes = d-tile pairs of x_hi vs W32l).
    12*256 cycles vs bf16's 8*512: 25% cheaper at bf16-level accuracy.
  - Scores run as fp8 DoubleRow with stationary [k_hi | k_hi] (stride-0
    plane broadcast) and moving [q_hi | q_lo]: full-precision q times fp8 k
    at 2x bf16 rate. Only the single k quantization (~2.4% rms) enters the
    softmax logits; measured end-to-end rel err ~9e-3 (gate 2e-2).
  - exp on ACT with scale = 1/(sqrt(D)*1024) (q,k both carry 32x).
  - PV stays bf16 (fp8 on either side measured 2-3e-2: over the gate).
Per-engine busy (cost model): PE ~69us, ACT (exp) ~66us, DVE ~55us.

Per-core pipeline (single interleaved emission). The schedule balances PE
against the ACT exp stream (64 x [128,1024] exps + per-instr access bubble
~= 68us, the hard floor): half 0 runs proj + scores + only 8 early PV
groups (window PV0_AT..) so PE tracks the exp rate; the other 24 PV groups
defer into half 1 as a ready reservoir (one pop per step, drained by step
23) that the list scheduler can also pull forward into any PE idle slot.
Their sT tiles keep dedicated SBUF bufs across the half boundary. Startup:
HBM loads ride the SP queue in consumption order (Wk, x0, Wq, x1, Wv, ...;
x6/x7 issued lazily from the weave so the vv XBAR transposes interleave),
and the first four score groups run 512 wide (j0 needs only k0/q0) so exps
start ~11us in. PSUM: proj(2 banks)+pss(4)+po0(2) in half 0; pss(4)+
po0(2)+po1(2) in half 1 (proj pool released first). Softmax denominator:
partials accumulate as bf16 adds on DVE (chains 0,1) / gpsimd (2,3; gpsimd
cannot touch PSUM) with late steps on DVE; chains collapse so the final
denominator is ones-matmuls over [c0, c2, sT31] closing right behind the
last (j-split) exp; half 0 folds mid-half-1 at FOLD0_AT (must stay >= the
last reservoir pop so po0 is complete). Epilogue: DVE reciprocal then
scalar_tensor_tensor (po * 1/32) * recip, output DMA in quarter columns.
TimelineSim: 90812 ns/core vs the bf16 predecessor's 112156 ns.
"""
from contextlib import ExitStack

import numpy as np
import ml_dtypes

import concourse.tile as tile
import concourse.mybir as mybir
from concourse import bacc
from concourse.bass_utils import run_bass_kernel_spmd

B, S, D, N, L = 4, 4096, 1024, 128, 128
NCORES = 8
SQ = B * S // NCORES      # 2048 queries per core
CCH = 512                 # projection chunk (tokens)
NCH = S // CCH            # 8 projection chunks
NKC = S // 128            # 32 kctx subchunks of 128
QH = 1024                 # query half processed per attention sweep
ND = D // 128             # 8 contraction tiles over D
NCHAIN = 4                # denominator partial chains
LAG_H1 = 3
SCALE = 1.0 / float(np.sqrt(D))
EXP_SCALE = SCALE / 1024.0   # q,k each carry a 32x from W32 host pre-scale

BF = mybir.dt.bfloat16
F32 = mybir.dt.float32
F8 = mybir.dt.float8e4
E4 = ml_dtypes.float8_e4m3
NWARM = 24
UPFRONT_X = 4
Q_ACT_COPY = True
PV0_AT = 14
POPS = 1
FOLD0_AT = 24
PACE = 26
EPI_QUARTERS = 1

DR = mybir.MatmulPerfMode.DoubleRow

# xt8 element offsets: [p, (c d two s)], two = (hi, lo)
XCH = ND * 2 * CCH        # elems per chunk per partition (8192)


def emit(nc, tc, ctx, xt, wcat, out):
    UPFRONT_X_ = UPFRONT_X
    persist = ctx.enter_context(tc.tile_pool(name="persist", bufs=1))
    zwarm = persist.tile([128, 128], BF, tag="zwarm")
    nc.gpsimd.memset(zwarm, 0.0)
    ones_mat = persist.tile([128, 128], BF, tag="ones_mat")
    nc.vector.memset(ones_mat, 1.0)

    xbig = persist.tile([128, NCH * XCH], F8, tag="xbig")
    # wcat: W32h [p, (m d n)] then W32l [p, (m d n)], m = (q, k, v)
    WSZ = 3 * ND * N
    wsb = persist.tile([128, 2 * WSZ], F8, tag="wsb")

    # HBM reads serialize at ~343 GB/s, so ALL loads ride one queue (SP) in
    # exactly the order the pipeline consumes them: Wq, x0, Wk, x1 (feeding
    # q0/k0/q1 and the first score group), then Wv and the remaining chunks.
    def xdma(c, parts=1):
        w = XCH // parts
        for s in range(parts):
            sl = slice(c * XCH + s * w, c * XCH + (s + 1) * w)
            nc.sync.dma_start(out=xbig[:, sl], in_=xt[:, sl])

    W1 = 2 * ND * N   # per-matrix (hi | lo) block
    half = XCH // 2

    def wdma(m):
        nc.sync.dma_start(out=wsb[:, m * W1:(m + 1) * W1],
                          in_=wcat[:, m * W1:(m + 1) * W1])

    # x6/x7 are issued lazily from the piece weave so the early vv XBAR
    # transposes (same SP queue) interleave with them instead of queuing
    # behind the whole x stream.
    wdma(1)
    xdma(0, parts=4)
    wdma(0)
    xdma(1, parts=2)
    wdma(2)
    for c in range(2, 2 + UPFRONT_X):
        xdma(c)

    def wh2(m, d):
        # stationary [128, 2, 128]: duplicated W32h_d planes (stride-0)
        off = m * W1 + d * N
        return wsb[:, off:off + N].unsqueeze(1).broadcast_to((128, 2, N))

    def wl2(m, t):
        # stationary [128, 2, 128]: planes (W32l_{2t}, W32l_{2t+1})
        off = m * W1 + ND * N + 2 * t * N
        return wsb[:, off:off + 2 * N].rearrange("p (two n) -> p two n",
                                                 two=2)

    def x12(c, d):
        # moving [128, 2, 512]: planes (x_hi_d, x_lo_d), contiguous
        off = c * XCH + d * 2 * CCH
        return xbig[:, off:off + 2 * CCH].rearrange(
            "p (two s) -> p two s", two=2)

    def x3(c, t):
        # moving [128, 2, 512]: planes (x_hi_{2t}, x_hi_{2t+1}), d-stride 1024
        off = c * XCH + 2 * t * 2 * CCH
        return xbig[:, off:off + 3 * CCH].rearrange(
            "p (d s) -> p d s", d=3)[:, 0::2, :]

    def proj_mm_a(ps, m, c):
        for d in range(ND):
            nc.tensor.matmul(ps, wh2(m, d), x12(c, d),
                             start=(d == 0), stop=False, perf_mode=DR)

    def proj_mm_b(ps, m, c):
        for t in range(ND // 2):
            nc.tensor.matmul(ps, wl2(m, t), x3(c, t),
                             start=False, stop=(t == ND // 2 - 1),
                             perf_mode=DR)

    kT8 = persist.tile([128, S], F8, tag="kT8")     # [n, kctx] fp8 (32x)
    vv = persist.tile([128, S], BF, tag="vv")       # 32 chunks [kctx128, l]
    qhl = persist.tile([128, 2 * SQ], F8, tag="qhl")  # per 512-q: (hi, lo)

    vtc_pool = ctx.enter_context(tc.tile_pool(name="vtc", bufs=3))

    # ---------------- attention pools (outer; proj pool nests inside) ----
    spool = ctx.enter_context(tc.tile_pool(name="sT", bufs=1))
    parts = ctx.enter_context(tc.tile_pool(name="parts", bufs=1))
    pss_pool = ctx.enter_context(tc.tile_pool(name="pss", bufs=2, space="PSUM"))
    po_pool = ctx.enter_context(tc.tile_pool(name="po", bufs=1, space="PSUM"))
    epi_sb = ctx.enter_context(tc.tile_pool(name="episb", bufs=2))

    proj_cm = tc.tile_pool(name="proj", bufs=2, space="PSUM")
    proj_ps = proj_cm.__enter__()
    po1_cm = tc.tile_pool(name="po1", bufs=1, space="PSUM")
    po1_holder = {"pool": None}

    # PE warmup while the first DMAs land (p-state ramp off critical path);
    # sized to end as x0 arrives so proj starts immediately at full rate
    for _ in range(NWARM):
        pwarm = proj_ps.tile([128, CCH], F32, tag="proj", name="pwarm")
        nc.tensor.matmul(pwarm[:, 0:128], zwarm, zwarm, start=True,
                         stop=True)

    vt_pending = []

    def flush_vt(n=None):
        while vt_pending and (n is None or len(vt_pending) > n):
            csl, vTc = vt_pending.pop(0)
            nc.sync.dma_start_transpose(
                out=vv[:, csl].rearrange("p (t q) -> p t q", t=CCH // 128),
                in_=vTc)

    # Each proj piece is emitted as two parts (pass12 / pass3+copies) so a
    # pending score group never waits behind a full 12-matmul chain.
    def piece_k(c):
        cell = {}
        csl = slice(c * CCH, (c + 1) * CCH)

        def a():
            if 2 + UPFRONT_X <= c + 2 < NCH:
                xdma(c + 2)
            cell["ps"] = proj_ps.tile([128, CCH], F32, tag="proj", name="pk")
            proj_mm_a(cell["ps"], 1, c)

        def b():
            proj_mm_b(cell["ps"], 1, c)
            # split the quantize so the first kctx subchunks unlock early
            h = CCH // 2
            nc.vector.tensor_copy(kT8[:, csl][:, 0:h], cell["ps"][:, 0:h])
            nc.vector.tensor_copy(kT8[:, csl][:, h:], cell["ps"][:, h:])
        return [a, b]

    def piece_v(c):
        cell = {}
        csl = slice(c * CCH, (c + 1) * CCH)

        def a():
            cell["ps"] = proj_ps.tile([128, CCH], F32, tag="proj", name="pv")
            proj_mm_a(cell["ps"], 2, c)

        def b():
            proj_mm_b(cell["ps"], 2, c)
            vTc = vtc_pool.tile([128, CCH], BF, tag="vTc", name=f"vTc{c % 3}")
            nc.vector.tensor_copy(vTc, cell["ps"])
            # defer the XBAR transpose issue one piece so it queues behind
            # the next x chunk's transfer on the SP DMA queue
            vt_pending.append((csl, vTc))
            flush_vt(1)
        return [a, b]

    def piece_q(c):
        cell = {}
        hi = slice(c * 2 * CCH, c * 2 * CCH + CCH)
        lo = slice(c * 2 * CCH + CCH, (c + 1) * 2 * CCH)

        def a():
            cell["ps"] = proj_ps.tile([128, CCH], F32, tag="proj", name="pq")
            proj_mm_a(cell["ps"], 0, c)

        def b():
            proj_mm_b(cell["ps"], 0, c)
            if c < 2 and Q_ACT_COPY:
                nc.scalar.activation(qhl[:, hi], cell["ps"],
                                     func=mybir.ActivationFunctionType.Copy)
            else:
                nc.vector.tensor_copy(qhl[:, hi], cell["ps"])
            nc.vector.tensor_tensor(out=qhl[:, lo], in0=cell["ps"],
                                    in1=qhl[:, hi],
                                    op=mybir.AluOpType.subtract)
        return [a, b]

    po = [None, None]
    part = {}
    sTs = {}
    NH0 = 8   # PV groups executed within half 0 (rest defer into half 1)
    LAG = LAG_H1   # PV trails its score by LAG steps (exp-gated pacing)

    def score_mm(h, i, j, out):
        ksl = slice(i * 128, (i + 1) * 128)
        kst = kT8[:, ksl].unsqueeze(1).broadcast_to((128, 2, 128))
        blk = h * 2 + j
        qmv = qhl[:, blk * 2 * CCH:(blk + 1) * 2 * CCH].rearrange(
            "p (two s) -> p two s", two=2)
        nc.tensor.matmul(out, kst, qmv, start=True, stop=True,
                         perf_mode=DR, skip_group_check=True)

    def sT_tag(h, i):
        # every h0 sT gets a dedicated buffer (PVs may run much later than
        # the score);  h1 rotates 8 bufs (PV trails by <= 8 there)
        return f"sTd{i}" if h == 0 else f"sTb{i % 8}"

    s512_ps = {}

    def emit_score512(h, i, j):
        # startup path: one j-block at a time (j0 needs only k0/q0); pss
        # tiles are shared by i-pairs so the pool rotation stays 2-deep
        key = (i // 2, j)
        if key not in s512_ps:
            s512_ps[key] = pss_pool.tile([128, QH], F32, tag="pss",
                                         name=f"pss512_{key}")
        ps = s512_ps[key][:, (i % 2) * CCH:(i % 2 + 1) * CCH]
        score_mm(h, i, j, ps)
        if j == 0:
            tg = sT_tag(h, i)
            sTs[h, i] = spool.tile([128, QH], BF, tag=tg, name=tg)
        sT = sTs[h, i]
        jsl = slice(j * (QH // 2), (j + 1) * (QH // 2))
        nc.scalar.activation(sT[:, jsl], ps,
                             func=mybir.ActivationFunctionType.Exp,
                             scale=EXP_SCALE)
        sTs[h, i] = sT
        if j == 1:
            emit_chain(h, i, sT)

    def emit_attn_score(h, i, split_exp=False):
        pss = pss_pool.tile([128, QH], F32, tag="pss")
        for j in range(2):
            score_mm(h, i, j,
                     pss[:, j * (QH // 2):(j + 1) * (QH // 2)])
        tg = sT_tag(h, i)
        sT = spool.tile([128, QH], BF, tag=tg, name=tg)
        sTs[h, i] = sT
        if split_exp:
            # last group of a half: per-j exps, each immediately folding the
            # denominator (3 ones-matmuls: c0, c2, this sT) so recip(j0)
            # starts one half-exp early
            prsR[h] = pss_pool.tile([128, QH], F32, tag="pss",
                                    name=f"prsR{h}")
            for j in range(2):
                jsl = slice(j * (QH // 2), (j + 1) * (QH // 2))
                nc.scalar.activation(sT[:, jsl], pss[:, jsl],
                                     func=mybir.ActivationFunctionType.Exp,
                                     scale=EXP_SCALE)
                emit_fold_mms(h, j, [part[h, 0], part[h, 2], sT],
                              start=True, stop=True)
            return
        nc.scalar.activation(sT, pss, func=mybir.ActivationFunctionType.Exp,
                             scale=EXP_SCALE)
        emit_chain(h, i, sT)

    def emit_chain(h, i, sT):
        # chains 2,3 ride gpsimd (SBUF-only engine, otherwise idle) except
        # near the end of a half so the fold is never gated on its slow
        # queue. The last 4 steps skip the vector add entirely: their sT
        # tiles feed the denominator fold matmuls directly.
        if i == NKC - 1:
            return  # sT(31) feeds the denominator fold directly
        ch = i % NCHAIN
        eng = nc.gpsimd if (ch >= 2 and i < NKC - 8) else nc.vector
        if i < NCHAIN:
            part[h, ch] = parts.tile([128, QH], BF, tag=f"part{h}_{ch}",
                                     name=f"part{h}_{ch}")
            eng.tensor_copy(part[h, ch], sT)
        elif i == NKC - 2:
            # j-split so c2's j0 half closes one half-add earlier
            for j in range(2):
                jsl = slice(j * (QH // 2), (j + 1) * (QH // 2))
                eng.tensor_add(part[h, ch][:, jsl], part[h, ch][:, jsl],
                               sT[:, jsl])
        else:
            eng.tensor_add(part[h, ch], part[h, ch], sT)

    def emit_attn_pv(h, i):
        ksl = slice(i * 128, (i + 1) * 128)
        if i == 0:
            # po0 shares half-0's psum budget with proj; po1's pool is
            # entered only after the proj pool releases its banks
            pool = po_pool if h == 0 else po1_holder["pool"]
            po[h] = pool.tile([128, QH], F32, tag=f"po{h}", name=f"po{h}")
        for j in range(2):
            jsl = slice(j * (QH // 2), (j + 1) * (QH // 2))
            nc.tensor.matmul(po[h][:, jsl], vv[:, ksl], sTs[h, i][:, jsl],
                             start=(i == 0), stop=(i == NKC - 1),
                             skip_group_check=True)

    prsR = [None, None]

    # The denominator fold is pure PE work: ones-matmuls accumulate the four
    # chain partials plus the last four sT tiles directly into prsR (every
    # partition gets the full denominator row), so no vector adds sit between
    # the final exp and the reciprocal.
    def fold_terms(h):
        return [part[h, ch] for ch in range(NCHAIN)]

    def emit_fold_mms(h, j, terms, start, stop):
        jsl = slice(j * (QH // 2), (j + 1) * (QH // 2))
        for t, term in enumerate(terms):
            nc.tensor.matmul(prsR[h][:, jsl], ones_mat, term[:, jsl],
                             start=(start and t == 0),
                             stop=(stop and t == len(terms) - 1),
                             skip_group_check=True)

    def emit_epi_finish(h):
        for j in range(2):
            jsl = slice(j * (QH // 2), (j + 1) * (QH // 2))
            recipB = epi_sb.tile([128, QH // 2], F32, tag="recipB")
            nc.vector.reciprocal(recipB, prsR[h][:, jsl])
            foall = epi_sb.tile([128, QH // 2], F32, tag="foall",
                                name=f"foall{j}")
            # v carries a 32x from W32v: out = (po * 1/32) * recip; quarter
            # columns pipeline the multiply with the output DMA
            for q in range(EPI_QUARTERS):
                w = (QH // 2) // EPI_QUARTERS
                qsl = slice(q * w, (q + 1) * w)
                base = h * QH + j * (QH // 2) + q * w
                nc.vector.scalar_tensor_tensor(
                    out=foall[:, qsl], in0=po[h][:, jsl][:, qsl],
                    scalar=1.0 / 32.0, in1=recipB[:, qsl],
                    op0=mybir.AluOpType.mult, op1=mybir.AluOpType.mult)
                nc.sync.dma_start(out=out[:, base:base + w],
                                  in_=foall[:, qsl])

    # ---------------- schedule ----------------
    # Half 0: proj piece-parts woven into the score steps (q0,k0,q1 up front
    # so the first exp lands early; k(c) hard-gated before step 4c; q2/q3
    # late). Only PV groups 0..NH0-1 run here (LAG-gated) — h0 stays just
    # under the ACT exp rate. The remaining PV groups defer into half 1 as a
    # ready reservoir the scheduler can pull into any PE idle slot.
    k_done = {"n": 0}

    def mk_k(c):
        a, b = piece_k(c)

        def b2():
            b()
            k_done["n"] = c + 1
        return [a, b2]

    rest = (mk_k(1) + piece_v(0) + piece_v(1) + mk_k(2) + piece_v(2)
            + mk_k(3) + piece_v(3) + mk_k(4) + piece_v(4) + mk_k(5)
            + piece_q(2) + piece_v(5) + mk_k(6) + piece_q(3) + piece_v(6)
            + mk_k(7) + piece_v(7))

    # Startup: k0/q0 feed four 512-wide j0 score groups (their exps start
    # while x1 is still in flight), then q1 unlocks the j1 halves.
    for f in mk_k(0) + piece_q(0):
        f()
    for i in range(4):
        emit_score512(0, i, 0)
    for f in piece_q(1):
        f()
    for i in range(4):
        emit_score512(0, i, 1)

    emitted = 0
    for i in range(4, NKC):
        need_k = i // 4 + 1
        while emitted < len(rest) and k_done["n"] < need_k:
            rest[emitted]()
            emitted += 1
        emit_attn_score(0, i)
        if PV0_AT <= i < NH0 + PV0_AT:
            emit_attn_pv(0, i - PV0_AT)
        while emitted < len(rest) and emitted * PACE < len(rest) * (i - 3):
            rest[emitted]()
            emitted += 1
    while emitted < len(rest):
        rest[emitted]()
        emitted += 1
    flush_vt(0)
    proj_cm.__exit__(None, None, None)
    po1_holder["pool"] = po1_cm.__enter__()

    # Half 1: scores with own PVs LAG-gated plus one deferred-h0 PV per
    # step (reservoir drains by step ~23); h0's fold/epilogue runs mid-
    # stream once po0 closes, so only h1's epilogue remains in the tail.
    # h0's chains collapse on DVE (mid-stream, off the critical path)
    for ch in (1, 2, 3):
        nc.vector.tensor_add(part[0, 0], part[0, 0], part[0, ch])
    pv0 = list(range(NH0, NKC))
    for s in range(NKC):
        emit_attn_score(1, s, split_exp=(s == NKC - 1))
        if s == NKC - NCHAIN:
            # c3 closed at step NKC-5: collapse into c0 off the tail path
            nc.vector.tensor_add(part[1, 0], part[1, 0], part[1, 3])
        elif s == NKC - 3:
            nc.vector.tensor_add(part[1, 0], part[1, 0], part[1, 1])
        if s >= LAG:
            emit_attn_pv(1, s - LAG)
        for _ in range(POPS):
            if pv0:
                emit_attn_pv(0, pv0.pop(0))
        if s == FOLD0_AT:
            prsR[0] = pss_pool.tile([128, QH], F32, tag="pss", name="prsR0")
            for j in range(2):
                emit_fold_mms(0, j, [part[0, 0], sTs[0, NKC - 1]],
                              start=True, stop=True)
            emit_epi_finish(0)
    for s in range(NKC - LAG, NKC):
        emit_attn_pv(1, s)
    emit_epi_finish(1)


def build_bass(iters=1):
    nc = bacc.Bacc()
    xt = nc.dram_tensor("xt_part", [128, NCH * XCH], F8,
                        kind="ExternalInput")
    wcat = nc.dram_tensor("wcat", [128, 2 * 3 * ND * N], F8,
                          kind="ExternalInput")
    out = nc.dram_tensor("out_part", [128, (SQ // 128) * L], F32,
                         kind="ExternalOutput")
    with tile.TileContext(nc) as tc:
        for _ in range(iters):
            with ExitStack() as ctx:
                emit(nc, tc, ctx, xt, wcat, out)
    nc.compile()
    return nc


def make_in_maps(x, Wq, Wk, Wv):
    # wcat[p, (m two d n)] = W32m.T[d*128+p, n], two = (hi, lo), W32 = 32*W
    blocks = []
    for W in (Wq, Wk, Wv):
        w32 = np.asarray(W, np.float32).T * 32.0          # [D, n]
        wh = w32.astype(E4)
        wl = (w32 - wh.astype(np.float32)).astype(E4)
        for w8 in (wh, wl):
            wt = w8.reshape(ND, 128, N)                   # [d, p, n]
            blocks.append(wt.transpose(1, 0, 2).reshape(128, ND * N))
    wcat = np.ascontiguousarray(
        np.concatenate(blocks, axis=1))                   # [128, 2*3*ND*N]
    x = np.asarray(x, np.float32)
    in_maps = []
    for c in range(NCORES):
        bb, h = c // 2, c % 2
        xb = x[bb]
        x_part = xb if h == 0 else np.concatenate([xb[SQ:], xb[:SQ]], axis=0)
        # xt[p, (c d two s)] = fp8 hi/lo of x_part[c*512+s, d*128+p]
        xr = x_part.reshape(NCH, CCH, ND, 128).transpose(3, 0, 2, 1)
        xh = xr.astype(E4)                                # [p, c, d, s]
        xl = (xr - xh.astype(np.float32)).astype(E4)
        xt_part = np.ascontiguousarray(
            np.stack([xh, xl], axis=3)                    # [p, c, d, two, s]
            .reshape(128, NCH * XCH))
        in_maps.append({"xt_part": xt_part, "wcat": wcat})
    return in_maps


def kernel(x, Wq, Wk, Wv):
    nc = build_bass()
    res = run_bass_kernel_spmd(nc, make_in_maps(x, Wq, Wk, Wv),
                               core_ids=list(range(NCORES)))
    out = np.empty((B, S, L), dtype=np.float32)
    for c in range(NCORES):
        bb, h = c // 2, c % 2
        # device layout out_dev[l, q]: final rows are columns
        out[bb, h * SQ:(h + 1) * SQ] = res.results[c]["out_part"].T
    return out
